# revision 8
# baseline (speedup 1.0000x reference)
"""MoDL (CNN denoiser + CG data-consistency MRI recon) on 8 Trainium2 NeuronCores.

Sharding: data-parallel over batch B=8, one batch element per core. The CG
dot-products are global over the batch; the loop is restructured as
Chronopoulos-Gear CG (apply A to r, keep s=Ap via the recurrence
s = w + beta*s) so each iteration needs ONE 3-scalar AllReduce placed right
after the matmul block instead of two serialized collectives, and every
vector update except "r" itself is off the critical path.

All FFTs are direct DFTs as PE matmuls (fp32r, full speed at free-dim>=256).
Each 1-D DFT stage uses the DATA as the stationary operand so every stage
output lands pre-transposed for the next stage (tall-split layout
[128, 2x256] throughout). The re/im result planes of each stage come from
ONE 512-wide matmul against a concatenated [F_a | F_b] moving operand
(half the matmul + LDWEIGHTS count of a 2x256 variant).

The coil sum (A r) accumulates per-coil in SBUF (no strided tensor_reduce),
and emission is software-pipelined (form of coil c+2 interleaved with the
mask/products of earlier coils) so in-order engine queues don't head-of-line
block the PE.

CNN: channels-on-partitions, shifted-window matmuls; 3x3 offsets packed in
pairs along the contraction axis. Layer-input strips live in two persistent
SBUF tiles whose zero borders are maintained with thin row memsets instead
of a full-tile memset per strip.
"""

from contextlib import ExitStack

import numpy as np

import concourse.bass as bass
import concourse.tile as tile
from concourse import bacc
from concourse import mybir
from concourse.bass_utils import run_bass_kernel_spmd
from concourse import bass_isa

FP = mybir.dt.float32
FPR = mybir.dt.float32r
AX = mybir.AxisListType
OP = mybir.AluOpType
AF = mybir.ActivationFunctionType

B, NCOIL, H, W = 8, 12, 256, 256
N_CG = 11
N_CORES = 8
HW = H * W


# ---------------------------------------------------------------- host prep

def _tall_split(plane):
    """[256,256] -> [128, 512]: col block t holds rows t*128..t*128+127."""
    return np.ascontiguousarray(
        plane.reshape(2, 128, 256).transpose(1, 0, 2).reshape(128, 512))


def _make_fcats():
    """Four [128,1024] concatenated DFT matrices.

    Layout per t-block (cols t*512..t*512+512): [A_t (256) | B_t (256)] where
    A feeds the re-half of the PSUM block and B the im-half.
      idx0 FWD pl=0: [ Fr | Fi]    idx1 FWD pl=1: [-Fi | Fr]
      idx2 INV pl=0: [ Fr |-Fi]    idx3 INV pl=1: [ Fi | Fr]
    """
    n = np.arange(256)
    Fm = np.exp(-2j * np.pi * np.outer(n, n) / 256) / 16.0
    fr = _tall_split(Fm.real.astype(np.float32))
    fi = _tall_split(Fm.imag.astype(np.float32))

    def cat(a, b):
        out = np.empty((128, 1024), np.float32)
        for t in range(2):
            out[:, t * 512:t * 512 + 256] = a[:, t * 256:(t + 1) * 256]
            out[:, t * 512 + 256:t * 512 + 512] = b[:, t * 256:(t + 1) * 256]
        return out
    return np.stack([cat(fr, fi), cat(-fi, fr), cat(fr, -fi), cat(fi, fr)])


def _prep_shared(w1, b1, w2, b2, w3, b3, w4, b4, w5, b5, lam):
    out = {}
    out["fcats"] = _make_fcats()
    out["wpack1"] = np.ascontiguousarray(
        np.asarray(w1, np.float32).transpose(2, 3, 1, 0).reshape(18, 64))

    def pack(wl):
        wl = np.asarray(wl, np.float32)
        cout = wl.shape[0]
        g = np.zeros((6, 128, cout), np.float32)
        for idy in range(3):
            g[idy * 2 + 0, 0:64] = wl[:, :, idy, 0].T    # dx=-1
            g[idy * 2 + 0, 64:128] = wl[:, :, idy, 1].T  # dx=0
            g[idy * 2 + 1, 0:64] = wl[:, :, idy, 2].T    # dx=+1
        return g
    out["wpack2"] = pack(w2)
    out["wpack3"] = pack(w3)
    out["wpack4"] = pack(w4)
    out["wpack5"] = pack(w5)
    for i, bl in enumerate((b1, b2, b3, b4, b5)):
        out[f"bias{i + 1}"] = np.asarray(bl, np.float32).reshape(-1, 1)
    out["lam"] = np.asarray(lam, np.float32).reshape(1, 1)
    return out


def _prep_core(atb_b, csm_re_b, csm_im_b, mask_b):
    out = {}
    pad = np.zeros((2, 258, 258), np.float32)
    pad[:, 1:257, 1:257] = atb_b
    im = np.empty((18, 256, 256), np.float32)
    for idy in range(3):
        for idx in range(3):
            o = idy * 3 + idx
            im[2 * o:2 * o + 2] = pad[:, idy:idy + 256, idx:idx + 256]
    out["im2col"] = np.ascontiguousarray(im.reshape(18, HW))
    out["atb_ts"] = np.stack([_tall_split(atb_b[0]), _tall_split(atb_b[1])])
    out["csm_re"] = np.ascontiguousarray(
        np.stack([_tall_split(csm_re_b[c]) for c in range(NCOIL)], axis=1)
        .reshape(128, NCOIL * 512))
    out["csm_im"] = np.ascontiguousarray(
        np.stack([_tall_split(csm_im_b[c]) for c in range(NCOIL)], axis=1)
        .reshape(128, NCOIL * 512))
    out["mask_ts"] = _tall_split(mask_b)
    return out


# ------------------------------------------------------------- bass program

def build_nc(n_cg=N_CG, n_coil=NCOIL, n_cores=N_CORES, cnn=True, evsem=True,
             use_cc=True):
    _uid = [0]

    def T(pool, shape, tag, dt=FP):
        _uid[0] += 1
        return pool.tile(shape, dt, tag=tag, name=f"{tag}_{_uid[0]}")

    nc = bass.Bass(num_devices=n_cores)
    group = [list(range(n_cores))]

    din = {}
    for name, shape in [
        ("im2col", [18, HW]), ("atb_ts", [2, 128, 512]),
        ("csm_re", [128, n_coil * 512]), ("csm_im", [128, n_coil * 512]),
        ("mask_ts", [128, 512]), ("fcats", [4, 128, 1024]),
        ("wpack1", [18, 64]), ("wpack2", [6, 128, 64]),
        ("wpack3", [6, 128, 64]), ("wpack4", [6, 128, 64]),
        ("wpack5", [6, 128, 2]),
        ("bias1", [64, 1]), ("bias2", [64, 1]), ("bias3", [64, 1]),
        ("bias4", [64, 1]), ("bias5", [2, 1]), ("lam", [1, 1]),
    ]:
        din[name] = nc.declare_dram_parameter(name, shape, FP, isOutput=False)
    dout = nc.declare_dram_parameter("out", [2, HW], FP, isOutput=True)

    acta = nc.dram_tensor("acta", [64, 256, 256], FP)
    actb = nc.dram_tensor("actb", [64, 256, 256], FP)
    h5d = nc.dram_tensor("h5d", [2, HW], FP)
    n_cc = n_cg
    ccin = [nc.dram_tensor(f"ccin{i}", [1, 8], FP) for i in range(n_cc)]
    cc_space = "Shared" if n_cores > 4 else "Local"
    ccout = [nc.dram_tensor(f"ccout{i}", [1, 8], FP, addr_space=cc_space)
             for i in range(n_cc)]

    with tile.TileContext(nc) as tc, ExitStack() as ctx:  # noqa: SIM117
        consts = ctx.enter_context(tc.tile_pool(name="consts", bufs=1))

        # ---- constants into SBUF (sync queue; CNN weights go on gpsimd
        # so they arrive first and gate nothing) ----
        fc = [T(consts, [128, 1024], f"fc{j}", FPR) for j in range(4)]
        for j in range(4):
            nc.gpsimd.dma_start(out=fc[j], in_=din["fcats"][j])
        csm_re = T(consts, [128, n_coil * 512], "csm_re")
        csm_im = T(consts, [128, n_coil * 512], "csm_im")
        nc.sync.dma_start(out=csm_re, in_=din["csm_re"][:])
        nc.sync.dma_start(out=csm_im, in_=din["csm_im"][:])
        mask = T(consts, [128, 512], "mask")
        nc.sync.dma_start(out=mask, in_=din["mask_ts"][:])
        lam128 = T(consts, [128, 1], "lam128")
        nc.sync.dma_start(out=lam128, in_=din["lam"][:].to_broadcast([128, 1]))
        ones128 = T(consts, [128, 1], "ones128")
        nc.vector.memset(ones128, 1.0)
        zrow = T(consts, [1, 8], "zrow")
        nc.vector.memset(zrow, 0.0)
        for i in range(n_cc):
            nc.sync.dma_start(out=ccin[i][:], in_=zrow)

        # =========================== CNN ===========================
        if cnn:
            with tc.tile_pool(name="cnnw", bufs=1) as cw, \
                 tc.tile_pool(name="cnnins", bufs=1) as cins, \
                 tc.tile_pool(name="cnnio", bufs=2) as cio, \
                 tc.tile_pool(name="cnnps", bufs=4, space="PSUM") as cps:
                wp1 = T(cw, [18, 64], "wp1", FPR)
                nc.gpsimd.dma_start(out=wp1, in_=din["wpack1"][:])
                bias = {}
                for l in (1, 2, 3, 4, 5):
                    cout = 2 if l == 5 else 64
                    bias[l] = T(cw, [cout, 1], f"b{l}")
                    nc.gpsimd.dma_start(out=bias[l], in_=din[f"bias{l}"][:])
                wp = {}
                for l in (2, 3, 4, 5):
                    cout = 2 if l == 5 else 64
                    wp[l] = [T(cw, [128, cout], f"w{l}g{g}", FPR)
                             for g in range(6)]
                    for g in range(6):
                        nc.gpsimd.dma_start(out=wp[l][g],
                                            in_=din[f"wpack{l}"][g])

                # ---- layer 1: K=18 im2col ----
                for s in range(16):
                    mv = T(cio, [18, 16 * 256], "l1mv", FPR)
                    nc.gpsimd.dma_start(
                        out=mv, in_=din["im2col"][:, s * 4096:(s + 1) * 4096])
                    ost = T(cio, [64, 16 * 256], "ostrip")
                    for k in range(8):
                        ps = T(cps, [64, 512], "ps")
                        nc.tensor.matmul(ps, wp1[:],
                                         mv[:, k * 512:(k + 1) * 512],
                                         start=True, stop=True)
                        nc.scalar.activation(ost[:, k * 512:(k + 1) * 512], ps,
                                             AF.Relu, bias=bias[1][:])
                    nc.sync.dma_start(
                        out=acta[:, s * 16:(s + 1) * 16, :],
                        in_=ost.rearrange("c (h w) -> c h w", w=256))

                # persistent double-buffered input strips; zero borders kept
                # valid across layers with thin row memsets only
                instile = [T(cins, [128, 18 * 258], f"ins{i}", FPR)
                           for i in range(2)]
                ins3t = [t.rearrange("p (r w) -> p r w", r=18)
                         for t in instile]
                for i in range(2):
                    nc.vector.memset(instile[i].bitcast(mybir.dt.uint32), 0)

                # ---- layers 2..5 ----
                for l, (src, dst) in zip(
                        (2, 3, 4, 5), [(acta, actb), (actb, acta),
                                       (acta, actb), (actb, None)]):
                    cout = 2 if l == 5 else 64
                    # strip 0 runs on tile 0 (needs row 0 zero), strip 15 on
                    # tile 1 (needs row 17 zero); both rows hold stale strip
                    # data from the previous layer.
                    if l > 2:
                        nc.vector.memset(
                            ins3t[0][:, 0:1, :].bitcast(mybir.dt.uint32), 0)
                        nc.vector.memset(
                            ins3t[1][:, 17:18, :].bitcast(mybir.dt.uint32), 0)
                    for s in range(16):
                        r0 = s * 16
                        ins3 = ins3t[s % 2]
                        ra = 0 if s > 0 else 1          # first valid ins row
                        rb = 18 if s < 15 else 17       # one past last valid
                        nc.gpsimd.dma_start(
                            out=ins3[0:64, ra:rb, 1:257],
                            in_=src[:, r0 - 1 + ra:r0 - 1 + rb, :])
                        nc.gpsimd.dma_start(
                            out=ins3[64:128, ra:rb, 0:256],
                            in_=src[:, r0 - 1 + ra:r0 - 1 + rb, :])
                        ost = T(cio, [cout, 16 * 256], "ostrip")
                        for k in range(8):
                            ps = T(cps, [cout, 512], "ps")
                            for idy in range(3):
                                for half in range(2):
                                    g = idy * 2 + half
                                    dxo = 2 if half else 0
                                    mvap = ins3[:,
                                                k * 2 + idy:k * 2 + idy + 2,
                                                dxo:dxo + 256]
                                    nc.tensor.matmul(
                                        ps, wp[l][g], mvap,
                                        start=(g == 0), stop=(g == 5))
                            nc.scalar.activation(
                                ost[:, k * 512:(k + 1) * 512], ps,
                                AF.Relu if l != 5 else AF.Identity,
                                bias=bias[l][:])
                        if l == 5:
                            nc.sync.dma_start(
                                out=h5d[:, s * 4096:(s + 1) * 4096], in_=ost)
                        else:
                            nc.sync.dma_start(
                                out=dst[:, s * 16:(s + 1) * 16, :],
                                in_=ost.rearrange("c (h w) -> c h w", w=256))

        # =========================== CG ===========================
        st = ctx.enter_context(tc.tile_pool(name="cgstate", bufs=1))
        wk = ctx.enter_context(tc.tile_pool(name="cgwork", bufs=2))
        wkc = ctx.enter_context(tc.tile_pool(name="cgcoil", bufs=3))
        pp = ctx.enter_context(tc.tile_pool(name="cgps", bufs=3, space="PSUM"))
        pq = ctx.enter_context(tc.tile_pool(name="cgps2", bufs=1,
                                            space="PSUM"))

        # vectors: [128,1024] = [re tall-split | im tall-split]
        r_t = [T(st, [128, 1024], "ra"), T(st, [128, 1024], "rb")]
        p2 = T(st, [128, 1024], "p2")
        s2 = T(st, [128, 1024], "s2")
        x2 = T(st, [128, 1024], "x2")
        w2 = T(st, [128, 1024], "w2")
        acc2 = T(st, [128, 1024], "acc2")
        lam1p = T(st, [128, 1], "lam1p")
        nc.vector.tensor_scalar_add(lam1p, lam128, 1.0)
        zero1 = T(st, [128, 1], "zero1")
        nc.vector.memset(zero1, 0.0)
        for t in (p2, s2, x2):
            nc.vector.memset(t, 0.0)

        # rhs = (1+lam)*atb + lam*h5 directly into r_t[0]
        for i in range(2):
            rsl = r_t[0][:, i * 512:(i + 1) * 512]
            nc.sync.dma_start(out=rsl, in_=din["atb_ts"][i])
            if cnn:
                h5p = T(wk, [128, 512], "ldh5")
                nc.sync.dma_start(
                    out=h5p,
                    in_=h5d[i].rearrange("(t p w) -> p t w",
                                         t=2, p=128, w=256))
                nc.vector.tensor_scalar(rsl, rsl, lam1p[:], None, op0=OP.mult)
                nc.vector.scalar_tensor_tensor(
                    rsl, h5p, lam128[:], rsl, op0=OP.mult, op1=OP.add)
            else:
                nc.vector.tensor_scalar(rsl, rsl, lam1p[:], None, op0=OP.mult)

        def split_planes(ps, out0, out1, dt_=FP):
            """De-interleave psum [128,1024] (m-blocks of [re|im]) into two
            contiguous [128,512] SBUF planes on the scalar engine (strided
            APs are slow on DVE but these 2D copies run at full rate)."""
            for m in range(2):
                nc.scalar.copy(out0[:, m * 256:(m + 1) * 256],
                               ps[:, m * 512:m * 512 + 256])
                nc.scalar.copy(out1[:, m * 256:(m + 1) * 256],
                               ps[:, m * 512 + 256:(m + 1) * 512])

        def dft_stage(inp, fwd):
            """inp: 2 FPR planes [128,512]; out psum [128,1024], m-blocks of
            [re(256) | im(256)]."""
            ps = T(pp, [128, 1024], "ps")
            k0 = 0 if fwd else 2
            for m in range(2):
                for t in range(2):
                    for pl in range(2):
                        nc.tensor.matmul(
                            ps[:, m * 512:(m + 1) * 512],
                            inp[pl][:, t * 256 + m * 128:
                                    t * 256 + m * 128 + 128],
                            fc[k0 + pl][:, t * 512:(t + 1) * 512],
                            start=(t == 0 and pl == 0),
                            stop=(t == 1 and pl == 1))
            return ps

        def emit_form(c, coil, r_cur):
            """coil_re = cr*r_re - ci*r_im ; coil_im = cr*r_im + ci*r_re"""
            cr = csm_re[:, c * 512:(c + 1) * 512]
            cim = csm_im[:, c * 512:(c + 1) * 512]
            rre = r_cur[:, 0:512]
            rim = r_cur[:, 512:1024]
            m1 = T(wk, [128, 512], "fm1")
            m2 = T(wk, [128, 512], "fm2")
            m3 = T(wk, [128, 512], "fm3")
            m4 = T(wk, [128, 512], "fm4")
            nc.vector.tensor_tensor(m1, cr, rre, op=OP.mult)
            nc.gpsimd.tensor_tensor(m2, cim, rim, op=OP.mult)
            nc.vector.tensor_tensor(m3, cr, rim, op=OP.mult)
            nc.gpsimd.tensor_tensor(m4, cim, rre, op=OP.mult)
            nc.vector.tensor_tensor(coil[0], m1, m2, op=OP.subtract)
            nc.vector.tensor_tensor(coil[1], m3, m4, op=OP.add)

        def emit_mask(ps2, Zt):
            ztmp0 = T(wk, [128, 512], "zc0")
            ztmp1 = T(wk, [128, 512], "zc1")
            split_planes(ps2, ztmp0, ztmp1)
            nc.vector.tensor_tensor(Zt[0], ztmp0, mask, op=OP.mult)
            nc.vector.tensor_tensor(Zt[1], ztmp1, mask, op=OP.mult)

        def emit_copies(ps1, Tt):
            split_planes(ps1, Tt[0], Tt[1])

        def emit_products(c, ps4, first):
            """acc_re += cr*U_re + ci*U_im ; acc_im += cr*U_im - ci*U_re

            GpSimd cannot read PSUM and strided APs are slow on DVE, so U is
            de-interleaved to SBUF planes on the scalar engine first."""
            cr = csm_re[:, c * 512:(c + 1) * 512]
            cim = csm_im[:, c * 512:(c + 1) * 512]
            are = acc2[:, 0:512]
            aim = acc2[:, 512:1024]
            ure = T(wk, [128, 512], "ure")
            uim = T(wk, [128, 512], "uim")
            split_planes(ps4, ure, uim)
            t1 = T(wk, [128, 512], "pt1")
            t2 = T(wk, [128, 512], "pt2")
            t3 = T(wk, [128, 512], "pt3")
            t4 = T(wk, [128, 512], "pt4")
            nc.vector.tensor_tensor(t1, cr, ure, op=OP.mult)
            nc.gpsimd.tensor_tensor(t2, cim, uim, op=OP.mult)
            nc.vector.tensor_tensor(t3, cr, uim, op=OP.mult)
            nc.gpsimd.tensor_tensor(t4, cim, ure, op=OP.mult)
            if first:
                nc.vector.tensor_tensor(are, t1, t2, op=OP.add)
                nc.vector.tensor_tensor(aim, t3, t4, op=OP.subtract)
            else:
                nc.vector.tensor_tensor(are, are, t1, op=OP.add)
                nc.vector.tensor_tensor(are, are, t2, op=OP.add)
                nc.vector.tensor_tensor(aim, aim, t3, op=OP.add)
                nc.vector.tensor_tensor(aim, aim, t4, op=OP.subtract)

        def emit_dot(a, b, part_slot):
            scrap = T(wk, [128, 1024], "dsc")
            nc.vector.scalar_tensor_tensor(
                scrap, a, 1.0, b, op0=OP.mult, op1=OP.mult,
                accum_out=part_slot)

        # ---------------- CG loop (Chronopoulos-Gear, unrolled) --------
        rho_old = None
        d_old = None
        alpha = None
        nalpha = None
        nab = None
        beta = None
        for it in range(n_cg):
            r_prev = r_t[(it + 1) % 2]
            r_cur = r_t[it % 2]
            parts = T(st, [128, 8], f"parts{it}")
            if it > 0:
                # r <- r_prev - alpha*(beta*s_old + w)  [s recurrence fused]
                rtmp = T(wk, [128, 1024], "rt")
                nc.vector.scalar_tensor_tensor(
                    rtmp, s2, nab[:], r_prev, op0=OP.mult, op1=OP.add)
                nc.vector.scalar_tensor_tensor(
                    r_cur, w2, nalpha[:], rtmp, op0=OP.mult, op1=OP.add)

            coils = {}

            def start_coil(c, _r=r_cur):
                coil = [T(wkc, [128, 512], f"coil{i}", FPR) for i in range(2)]
                emit_form(c, coil, _r)
                coils[c] = coil

            start_coil(0)
            if it > 0:
                # deferred recurrences (off the critical path):
                # s <- w + beta*s ; p <- r_prev + beta*p ; x <- x + alpha*p
                nc.vector.scalar_tensor_tensor(
                    s2, s2, beta[:], w2, op0=OP.mult, op1=OP.add)
                nc.vector.scalar_tensor_tensor(
                    p2, p2, beta[:], r_prev, op0=OP.mult, op1=OP.add)
                nc.vector.scalar_tensor_tensor(
                    x2, p2, alpha[:], x2, op0=OP.mult, op1=OP.add)
            # dots that only need r and s_{it-1}
            emit_dot(r_cur, r_cur, parts[:, 0:1])
            emit_dot(r_cur, s2, parts[:, 2:3])
            start_coil(1)

            for c in range(n_coil):
                Tt = [T(wk, [128, 512], f"T{i}", FPR) for i in range(2)]
                Zt = [T(wk, [128, 512], f"Z{i}", FPR) for i in range(2)]
                Ut = [T(wk, [128, 512], f"U{i}", FPR) for i in range(2)]
                ps1 = dft_stage(coils[c], True)
                emit_copies(ps1, Tt)
                ps2 = dft_stage(Tt, True)
                if c + 2 < n_coil:
                    start_coil(c + 2)
                emit_mask(ps2, Zt)
                ps3 = dft_stage(Zt, False)
                emit_copies(ps3, Ut)
                ps4 = dft_stage(Ut, False)
                emit_products(c, ps4, first=(c == 0))
                del coils[c]

            # mu_acc dot; w = lam*r + acc early (overlaps the collective)
            emit_dot(r_cur, acc2, parts[:, 1:2])
            nc.vector.scalar_tensor_tensor(
                w2, r_cur, lam128[:], acc2, op0=OP.mult, op1=OP.add)
            # pre-reduce the three partial sums across partitions on the PE
            psr = T(pq, [1, 8], "psr")
            nc.tensor.matmul(psr[0:1, 0:3], ones128[:, 0:1], parts[:, 0:3],
                             start=True, stop=True)
            small = T(st, [1, 8], f"small{it}")
            nc.scalar.copy(small[0:1, 0:3], psr[0:1, 0:3])
            nc.gpsimd.dma_start(out=ccin[it][0:1, 0:3], in_=small[0:1, 0:3])
            if use_cc:
                nc.gpsimd.collective_compute(
                    "AllReduce", OP.add, replica_groups=group,
                    ins=[ccin[it][:]], outs=[ccout[it][:]])
                src_cc = ccout[it]
            else:
                src_cc = ccin[it]
            bc = T(st, [128, 8], f"bc{it}")
            nc.gpsimd.dma_start(out=bc, in_=src_cc[:].to_broadcast([128, 8]))

            # scalar recurrences on [128,1] replicated values
            rho = bc[:, 0:1]
            mu_acc = bc[:, 1:2]
            nu = bc[:, 2:3]
            mu = T(st, [128, 1], f"mu{it}")
            nc.vector.scalar_tensor_tensor(
                mu, rho, lam128[:], mu_acc, op0=OP.mult, op1=OP.add)
            if it == 0:
                beta = zero1
                dd = mu
            else:
                rro = T(st, [128, 1], f"rro{it}")
                nc.vector.reciprocal(rro, rho_old)
                beta = T(st, [128, 1], f"beta{it}")
                nc.vector.tensor_tensor(beta, rro, rho, op=OP.mult)
                # d = mu + beta*(2*nu + beta*d_old)
                q1 = T(st, [128, 1], f"q1{it}")
                nc.vector.tensor_tensor(q1, beta, d_old, op=OP.mult)
                q2 = T(st, [128, 1], f"q2{it}")
                nc.vector.scalar_tensor_tensor(
                    q2, nu, 2.0, q1, op0=OP.mult, op1=OP.add)
                q3 = T(st, [128, 1], f"q3{it}")
                nc.vector.tensor_tensor(q3, beta, q2, op=OP.mult)
                dd = T(st, [128, 1], f"dd{it}")
                nc.vector.tensor_tensor(dd, q3, mu, op=OP.add)
            rrd = T(st, [128, 1], f"rrd{it}")
            nc.vector.reciprocal(rrd, dd)
            alpha = T(st, [128, 1], f"al{it}")
            nc.vector.tensor_tensor(alpha, rrd, rho, op=OP.mult)
            nalpha = T(st, [128, 1], f"nal{it}")
            nc.vector.tensor_scalar_mul(nalpha, alpha, -1.0)
            nab = T(st, [128, 1], f"nab{it}")
            nc.vector.tensor_tensor(nab, nalpha, beta, op=OP.mult)
            rho_old = rho
            d_old = dd

            if it == n_cg - 1:
                # final p, x updates: p10 = r10 + beta10*p9 ; x11 += a10*p10
                nc.vector.scalar_tensor_tensor(
                    p2, p2, beta[:], r_cur, op0=OP.mult, op1=OP.add)
                nc.vector.scalar_tensor_tensor(
                    x2, p2, alpha[:], x2, op0=OP.mult, op1=OP.add)

        # ---- output: x tall-split -> natural [2, 256*256] ----
        for i in range(2):
            nc.sync.dma_start(
                out=dout[i].rearrange("(t p w) -> p t w", t=2, p=128, w=256),
                in_=x2[:, i * 512:(i + 1) * 512].rearrange(
                    "p (t w) -> p t w", t=2))

    if evsem:
        import bass_rust as _bass_rust
        _bass_rust.generate_event_semaphores(nc)
        mybir.codegen_inst_isa_subclasses(nc)
    return nc


# ------------------------------------------------------------------ runner

_CACHE = {}


def _get_nc(key=(N_CG, NCOIL, N_CORES, True)):
    if key not in _CACHE:
        _CACHE[key] = build_nc(*key)
    return _CACHE[key]


def make_in_maps(inputs):
    shared = _prep_shared(
        inputs["w1"], inputs["b1"], inputs["w2"], inputs["b2"], inputs["w3"],
        inputs["b3"], inputs["w4"], inputs["b4"], inputs["w5"], inputs["b5"],
        inputs["lam"])
    in_maps = []
    for b in range(N_CORES):
        m = dict(shared)
        m.update(_prep_core(
            np.asarray(inputs["atb"][b], np.float32),
            np.asarray(inputs["csm_real"][b], np.float32),
            np.asarray(inputs["csm_imag"][b], np.float32),
            np.asarray(inputs["mask"][b], np.float32)))
        in_maps.append(m)
    return in_maps


def run(inputs, trace=False, **kw):
    nc = _get_nc()
    in_maps = make_in_maps(inputs)
    res = run_bass_kernel_spmd(nc, in_maps, core_ids=list(range(N_CORES)),
                               trace=trace, **kw)
    out = np.stack([np.asarray(r["out"]).reshape(2, 256, 256)
                    for r in res.results]).astype(np.float32)
    return out, res


def kernel(**inputs):
    out, _ = run(inputs, trace=False)
    return out


# revision 10
# speedup vs baseline: 1.1757x; 1.1757x over previous
"""MoDL (CNN denoiser + CG data-consistency MRI recon) on 8 Trainium2 NeuronCores.

Sharding: data-parallel over batch B=8, one batch element per core. The CG
dot-products are global over the batch; the loop is restructured as
Chronopoulos-Gear CG (apply A to r, keep s=Ap via the recurrence
s = w + beta*s) so each iteration needs ONE tiny AllReduce placed right
after the matmul block instead of two serialized collectives, and every
vector update except "r" itself is off the critical path.

All FFTs are direct DFTs as PE matmuls (fp32r, full speed at free-dim>=256).
Each 1-D DFT stage uses the DATA as the stationary operand so every stage
output lands pre-transposed for the next stage (tall-split layout
[128, 2x256] throughout); re/im planes go to separate contiguous PSUM tiles
so every DVE op downstream is a fast contiguous [128,512] op (strided APs
and PSUM de-interleaving both measured much slower).

The coil sum (A r) accumulates per-coil into separate acc_re/acc_im SBUF
tiles (independent vector/gpsimd dependency chains, no strided
tensor_reduce), and emission is software-pipelined (form of coil c+2
interleaved with the mask/products of earlier coils) so the in-order engine
queues never head-of-line block the PE.

CNN: channels-on-partitions, shifted-window matmuls; 3x3 offsets packed in
pairs along the contraction axis. Layer-input strips live in three rotating
persistent SBUF tiles whose zero borders are maintained with thin row
memsets instead of a full-tile memset per strip.
"""

from contextlib import ExitStack

import numpy as np

import concourse.bass as bass
import concourse.tile as tile
from concourse import bacc
from concourse import mybir
from concourse.bass_utils import run_bass_kernel_spmd
from concourse import bass_isa

FP = mybir.dt.float32
FPR = mybir.dt.float32r
AX = mybir.AxisListType
OP = mybir.AluOpType
AF = mybir.ActivationFunctionType

B, NCOIL, H, W = 8, 12, 256, 256
N_CG = 11
N_CORES = 8
HW = H * W


# ---------------------------------------------------------------- host prep

def _tall_split(plane):
    """[256,256] -> [128, 512]: col block t holds rows t*128..t*128+127."""
    return np.ascontiguousarray(
        plane.reshape(2, 128, 256).transpose(1, 0, 2).reshape(128, 512))


def _make_fmats():
    n = np.arange(256)
    Fm = np.exp(-2j * np.pi * np.outer(n, n) / 256) / 16.0
    fr = Fm.real.astype(np.float32)
    fi = Fm.imag.astype(np.float32)
    return np.stack([_tall_split(fr), _tall_split(fi), _tall_split(-fi)])


def _prep_shared(w1, b1, w2, b2, w3, b3, w4, b4, w5, b5, lam):
    out = {}
    out["fmats"] = _make_fmats()
    out["wpack1"] = np.ascontiguousarray(
        np.asarray(w1, np.float32).transpose(2, 3, 1, 0).reshape(18, 64))

    def pack(wl):
        wl = np.asarray(wl, np.float32)
        cout = wl.shape[0]
        g = np.zeros((6, 128, cout), np.float32)
        for idy in range(3):
            g[idy * 2 + 0, 0:64] = wl[:, :, idy, 0].T    # dx=-1
            g[idy * 2 + 0, 64:128] = wl[:, :, idy, 1].T  # dx=0
            g[idy * 2 + 1, 0:64] = wl[:, :, idy, 2].T    # dx=+1
        return g
    out["wpack2"] = pack(w2)
    out["wpack3"] = pack(w3)
    out["wpack4"] = pack(w4)
    out["wpack5"] = pack(w5)
    for i, bl in enumerate((b1, b2, b3, b4, b5)):
        out[f"bias{i + 1}"] = np.asarray(bl, np.float32).reshape(-1, 1)
    out["lam"] = np.asarray(lam, np.float32).reshape(1, 1)
    return out


def _prep_core(atb_b, csm_re_b, csm_im_b, mask_b):
    out = {}
    pad = np.zeros((2, 258, 258), np.float32)
    pad[:, 1:257, 1:257] = atb_b
    im = np.empty((18, 256, 256), np.float32)
    for idy in range(3):
        for idx in range(3):
            o = idy * 3 + idx
            im[2 * o:2 * o + 2] = pad[:, idy:idy + 256, idx:idx + 256]
    out["im2col"] = np.ascontiguousarray(im.reshape(18, HW))
    out["atb_ts"] = np.stack([_tall_split(atb_b[0]), _tall_split(atb_b[1])])
    out["csm_re"] = np.ascontiguousarray(
        np.stack([_tall_split(csm_re_b[c]) for c in range(NCOIL)], axis=1)
        .reshape(128, NCOIL * 512))
    out["csm_im"] = np.ascontiguousarray(
        np.stack([_tall_split(csm_im_b[c]) for c in range(NCOIL)], axis=1)
        .reshape(128, NCOIL * 512))
    out["mask_ts"] = _tall_split(mask_b)
    return out


# ------------------------------------------------------------- bass program

def build_nc(n_cg=N_CG, n_coil=NCOIL, n_cores=N_CORES, cnn=True, evsem=True,
             use_cc=True):
    _uid = [0]

    def T(pool, shape, tag, dt=FP):
        _uid[0] += 1
        return pool.tile(shape, dt, tag=tag, name=f"{tag}_{_uid[0]}")

    nc = bass.Bass(num_devices=n_cores)
    group = [list(range(n_cores))]

    din = {}
    for name, shape in [
        ("im2col", [18, HW]), ("atb_ts", [2, 128, 512]),
        ("csm_re", [128, n_coil * 512]), ("csm_im", [128, n_coil * 512]),
        ("mask_ts", [128, 512]), ("fmats", [3, 128, 512]),
        ("wpack1", [18, 64]), ("wpack2", [6, 128, 64]),
        ("wpack3", [6, 128, 64]), ("wpack4", [6, 128, 64]),
        ("wpack5", [6, 128, 2]),
        ("bias1", [64, 1]), ("bias2", [64, 1]), ("bias3", [64, 1]),
        ("bias4", [64, 1]), ("bias5", [2, 1]), ("lam", [1, 1]),
    ]:
        din[name] = nc.declare_dram_parameter(name, shape, FP, isOutput=False)
    dout = nc.declare_dram_parameter("out", [2, HW], FP, isOutput=True)

    acta = nc.dram_tensor("acta", [64, 256, 256], FP)
    actb = nc.dram_tensor("actb", [64, 256, 256], FP)
    h5d = nc.dram_tensor("h5d", [2, HW], FP)
    n_cc = n_cg
    ccin = [nc.dram_tensor(f"ccin{i}", [1, 8], FP) for i in range(n_cc)]
    cc_space = "Shared" if n_cores > 4 else "Local"
    ccout = [nc.dram_tensor(f"ccout{i}", [1, 8], FP, addr_space=cc_space)
             for i in range(n_cc)]

    with tile.TileContext(nc) as tc, ExitStack() as ctx:  # noqa: SIM117
        consts = ctx.enter_context(tc.tile_pool(name="consts", bufs=1))

        # ---- constants into SBUF (sync queue except casts; CNN weights are
        # first on the gpsimd queue so they gate nothing) ----
        fm = [T(consts, [128, 512], f"fm{j}", FPR) for j in range(3)]
        for j in range(3):
            nc.gpsimd.dma_start(out=fm[j], in_=din["fmats"][j])
        csm_re = T(consts, [128, n_coil * 512], "csm_re")
        csm_im = T(consts, [128, n_coil * 512], "csm_im")
        nc.sync.dma_start(out=csm_re, in_=din["csm_re"][:])
        nc.sync.dma_start(out=csm_im, in_=din["csm_im"][:])
        mask = T(consts, [128, 512], "mask")
        nc.sync.dma_start(out=mask, in_=din["mask_ts"][:])
        lam128 = T(consts, [128, 1], "lam128")
        nc.sync.dma_start(out=lam128, in_=din["lam"][:].to_broadcast([128, 1]))
        ones128 = T(consts, [128, 1], "ones128")
        nc.vector.memset(ones128, 1.0)
        zrow = T(consts, [1, 8], "zrow")
        nc.vector.memset(zrow, 0.0)
        for i in range(n_cc):
            nc.sync.dma_start(out=ccin[i][:], in_=zrow)

        # =========================== CNN ===========================
        if cnn:
            with tc.tile_pool(name="cnnw", bufs=1) as cw, \
                 tc.tile_pool(name="cnnins", bufs=1) as cins, \
                 tc.tile_pool(name="cnnio", bufs=2) as cio, \
                 tc.tile_pool(name="cnnps", bufs=4, space="PSUM") as cps:
                wp1 = T(cw, [18, 64], "wp1", FPR)
                nc.gpsimd.dma_start(out=wp1, in_=din["wpack1"][:])
                bias = {}
                for l in (1, 2, 3, 4, 5):
                    cout = 2 if l == 5 else 64
                    bias[l] = T(cw, [cout, 1], f"b{l}")
                    nc.gpsimd.dma_start(out=bias[l], in_=din[f"bias{l}"][:])
                wp = {}
                for l in (2, 3, 4, 5):
                    cout = 2 if l == 5 else 64
                    wp[l] = [T(cw, [128, cout], f"w{l}g{g}", FPR)
                             for g in range(6)]
                    for g in range(6):
                        nc.gpsimd.dma_start(out=wp[l][g],
                                            in_=din[f"wpack{l}"][g])

                # ---- layer 1: K=18 im2col ----
                for s in range(16):
                    mv = T(cio, [18, 16 * 256], "l1mv", FPR)
                    nc.gpsimd.dma_start(
                        out=mv, in_=din["im2col"][:, s * 4096:(s + 1) * 4096])
                    ost = T(cio, [64, 16 * 256], "ostrip")
                    for k in range(8):
                        ps = T(cps, [64, 512], "ps")
                        nc.tensor.matmul(ps, wp1[:],
                                         mv[:, k * 512:(k + 1) * 512],
                                         start=True, stop=True)
                        nc.scalar.activation(ost[:, k * 512:(k + 1) * 512], ps,
                                             AF.Relu, bias=bias[1][:])
                    nc.sync.dma_start(
                        out=acta[:, s * 16:(s + 1) * 16, :],
                        in_=ost.rearrange("c (h w) -> c h w", w=256))

                # three rotating persistent input-strip tiles; zero borders
                # maintained with thin row memsets only
                instile = [T(cins, [128, 18 * 258], f"ins{i}", FPR)
                           for i in range(3)]
                ins3t = [t.rearrange("p (r w) -> p r w", r=18)
                         for t in instile]
                for i in range(3):
                    nc.vector.memset(instile[i].bitcast(mybir.dt.uint32), 0)

                # ---- layers 2..5 ----
                for l, (src, dst) in zip(
                        (2, 3, 4, 5), [(acta, actb), (actb, acta),
                                       (acta, actb), (actb, None)]):
                    cout = 2 if l == 5 else 64
                    if l > 2:
                        # strip 0 needs row 0 zero; dirty from prev layer
                        nc.vector.memset(
                            ins3t[0][:, 0:1, :].bitcast(mybir.dt.uint32), 0)
                    for s in range(16):
                        r0 = s * 16
                        ins3 = ins3t[s % 3]
                        ra = 0 if s > 0 else 1          # first valid ins row
                        rb = 18 if s < 15 else 17       # one past last valid
                        if s == 15:
                            # row 17 must be zero; dirty from strip 12
                            nc.vector.memset(
                                ins3[:, 17:18, :].bitcast(mybir.dt.uint32), 0)
                        nc.gpsimd.dma_start(
                            out=ins3[0:64, ra:rb, 1:257],
                            in_=src[:, r0 - 1 + ra:r0 - 1 + rb, :])
                        nc.gpsimd.dma_start(
                            out=ins3[64:128, ra:rb, 0:256],
                            in_=src[:, r0 - 1 + ra:r0 - 1 + rb, :])
                        ost = T(cio, [cout, 16 * 256], "ostrip")
                        for k in range(8):
                            ps = T(cps, [cout, 512], "ps")
                            for idy in range(3):
                                for half in range(2):
                                    g = idy * 2 + half
                                    dxo = 2 if half else 0
                                    mvap = ins3[:,
                                                k * 2 + idy:k * 2 + idy + 2,
                                                dxo:dxo + 256]
                                    nc.tensor.matmul(
                                        ps, wp[l][g], mvap,
                                        start=(g == 0), stop=(g == 5))
                            nc.scalar.activation(
                                ost[:, k * 512:(k + 1) * 512], ps,
                                AF.Relu if l != 5 else AF.Identity,
                                bias=bias[l][:])
                        if l == 5:
                            nc.sync.dma_start(
                                out=h5d[:, s * 4096:(s + 1) * 4096], in_=ost)
                        else:
                            nc.sync.dma_start(
                                out=dst[:, s * 16:(s + 1) * 16, :],
                                in_=ost.rearrange("c (h w) -> c h w", w=256))

        # =========================== CG ===========================
        st = ctx.enter_context(tc.tile_pool(name="cgstate", bufs=1))
        wk = ctx.enter_context(tc.tile_pool(name="cgwork", bufs=2))
        wkc = ctx.enter_context(tc.tile_pool(name="cgcoil", bufs=3))
        pp = ctx.enter_context(tc.tile_pool(name="cgps", bufs=3, space="PSUM"))
        pq = ctx.enter_context(tc.tile_pool(name="cgps2", bufs=1,
                                            space="PSUM"))

        # vectors: [128,1024] = [re tall-split | im tall-split]
        r_t = [T(st, [128, 1024], "ra"), T(st, [128, 1024], "rb")]
        p2 = T(st, [128, 1024], "p2")
        s2 = T(st, [128, 1024], "s2")
        x2 = T(st, [128, 1024], "x2")
        w2 = T(st, [128, 1024], "w2")
        acc_re = T(st, [128, 512], "acc_re")
        acc_im = T(st, [128, 512], "acc_im")
        lam1p = T(st, [128, 1], "lam1p")
        nc.vector.tensor_scalar_add(lam1p, lam128, 1.0)
        zero1 = T(st, [128, 1], "zero1")
        nc.vector.memset(zero1, 0.0)
        for t in (p2, s2, x2):
            nc.vector.memset(t, 0.0)

        # rhs = (1+lam)*atb + lam*h5 directly into r_t[0]
        for i in range(2):
            rsl = r_t[0][:, i * 512:(i + 1) * 512]
            nc.sync.dma_start(out=rsl, in_=din["atb_ts"][i])
            if cnn:
                h5p = T(wk, [128, 512], "ldh5")
                nc.sync.dma_start(
                    out=h5p,
                    in_=h5d[i].rearrange("(t p w) -> p t w",
                                         t=2, p=128, w=256))
                nc.vector.tensor_scalar(rsl, rsl, lam1p[:], None, op0=OP.mult)
                nc.vector.scalar_tensor_tensor(
                    rsl, h5p, lam128[:], rsl, op0=OP.mult, op1=OP.add)
            else:
                nc.vector.tensor_scalar(rsl, rsl, lam1p[:], None, op0=OP.mult)

        FWD = ((0, 2), (1, 0))   # re: Xr*Fr + Xi*(-Fi); im: Xr*Fi + Xi*Fr
        INV = ((0, 1), (2, 0))   # re: Xr*Fr + Xi*Fi;    im: Xr*(-Fi) + Xi*Fr

        def F(j, t):
            return fm[j][:, t * 256:(t + 1) * 256]

        def dft_stage(inp, fwd):
            """inp: 2 FPR planes [128,512]; out: 2 psum planes [128,512]."""
            combo = FWD if fwd else INV
            psA = T(pp, [128, 512], "psA")
            psB = T(pp, [128, 512], "psB")
            for m in range(2):
                for t in range(2):
                    for pl in range(2):
                        lt = inp[pl][:, t * 256 + m * 128:
                                     t * 256 + m * 128 + 128]
                        fst = (t == 0 and pl == 0)
                        lst = (t == 1 and pl == 1)
                        nc.tensor.matmul(psA[:, m * 256:(m + 1) * 256], lt,
                                         F(combo[0][pl], t),
                                         start=fst, stop=lst)
                        nc.tensor.matmul(psB[:, m * 256:(m + 1) * 256], lt,
                                         F(combo[1][pl], t),
                                         start=fst, stop=lst)
            return psA, psB

        def emit_form(c, coil, r_cur):
            """coil_re = cr*r_re - ci*r_im ; coil_im = cr*r_im + ci*r_re"""
            cr = csm_re[:, c * 512:(c + 1) * 512]
            cim = csm_im[:, c * 512:(c + 1) * 512]
            rre = r_cur[:, 0:512]
            rim = r_cur[:, 512:1024]
            m1 = T(wk, [128, 512], "fm1")
            m2 = T(wk, [128, 512], "fm2")
            m3 = T(wk, [128, 512], "fm3")
            m4 = T(wk, [128, 512], "fm4")
            nc.vector.tensor_tensor(m1, cr, rre, op=OP.mult)
            nc.gpsimd.tensor_tensor(m2, cim, rim, op=OP.mult)
            nc.vector.tensor_tensor(m3, cr, rim, op=OP.mult)
            nc.gpsimd.tensor_tensor(m4, cim, rre, op=OP.mult)
            nc.vector.tensor_tensor(coil[0], m1, m2, op=OP.subtract)
            nc.gpsimd.tensor_tensor(coil[1], m3, m4, op=OP.add)

        def emit_mask(ps2, Zt):
            nc.vector.tensor_tensor(Zt[0], ps2[0], mask, op=OP.mult)
            nc.vector.tensor_tensor(Zt[1], ps2[1], mask, op=OP.mult)

        def emit_copies(ps1, Tt):
            nc.scalar.copy(Tt[0], ps1[0])
            nc.scalar.copy(Tt[1], ps1[1])

        def emit_products(c, ps4, first):
            """acc_re += cr*U_re + ci*U_im ; acc_im += cr*U_im - ci*U_re

            Product mults on vector (gpsimd cannot read PSUM); the re-chain
            accumulates on vector, the im-chain on gpsimd (separate acc
            tiles keep the chains independent)."""
            cr = csm_re[:, c * 512:(c + 1) * 512]
            cim = csm_im[:, c * 512:(c + 1) * 512]
            t1 = T(wk, [128, 512], "pt1")
            t2 = T(wk, [128, 512], "pt2")
            t3 = T(wk, [128, 512], "pt3")
            t4 = T(wk, [128, 512], "pt4")
            nc.vector.tensor_tensor(t1, cr, ps4[0], op=OP.mult)
            nc.vector.tensor_tensor(t2, cim, ps4[1], op=OP.mult)
            nc.vector.tensor_tensor(t3, cr, ps4[1], op=OP.mult)
            nc.vector.tensor_tensor(t4, cim, ps4[0], op=OP.mult)
            if first:
                nc.vector.tensor_tensor(acc_re, t1, t2, op=OP.add)
                nc.gpsimd.tensor_tensor(acc_im, t3, t4, op=OP.subtract)
            else:
                nc.vector.tensor_tensor(acc_re, acc_re, t1, op=OP.add)
                nc.vector.tensor_tensor(acc_re, acc_re, t2, op=OP.add)
                nc.gpsimd.tensor_tensor(acc_im, acc_im, t3, op=OP.add)
                nc.gpsimd.tensor_tensor(acc_im, acc_im, t4, op=OP.subtract)

        def emit_dot(a, b, part_slot):
            scrap = T(wk, [128, 1024], "dsc")
            nc.vector.scalar_tensor_tensor(
                scrap, a, 1.0, b, op0=OP.mult, op1=OP.mult,
                accum_out=part_slot)

        def emit_dot512(a, b, part_slot):
            scrap = T(wk, [128, 512], "dsc5")
            nc.vector.scalar_tensor_tensor(
                scrap, a, 1.0, b, op0=OP.mult, op1=OP.mult,
                accum_out=part_slot)

        # ---------------- CG loop (Chronopoulos-Gear, unrolled) --------
        rho_old = None
        d_old = None
        alpha = None
        nalpha = None
        nab = None
        beta = None
        for it in range(n_cg):
            r_prev = r_t[(it + 1) % 2]
            r_cur = r_t[it % 2]
            parts = T(st, [128, 8], f"parts{it}")
            if it > 0:
                # r <- r_prev - alpha*(beta*s_old + w)  [s recurrence fused]
                rtmp = T(wk, [128, 1024], "rt")
                nc.vector.scalar_tensor_tensor(
                    rtmp, s2, nab[:], r_prev, op0=OP.mult, op1=OP.add)
                nc.vector.scalar_tensor_tensor(
                    r_cur, w2, nalpha[:], rtmp, op0=OP.mult, op1=OP.add)

            coils = {}

            def start_coil(c, _r=r_cur):
                coil = [T(wkc, [128, 512], f"coil{i}", FPR) for i in range(2)]
                emit_form(c, coil, _r)
                coils[c] = coil

            start_coil(0)
            if it > 0:
                # deferred recurrences (off the critical path):
                # s <- w + beta*s ; p <- r_prev + beta*p ; x <- x + alpha*p
                nc.vector.scalar_tensor_tensor(
                    s2, s2, beta[:], w2, op0=OP.mult, op1=OP.add)
                nc.vector.scalar_tensor_tensor(
                    p2, p2, beta[:], r_prev, op0=OP.mult, op1=OP.add)
                nc.vector.scalar_tensor_tensor(
                    x2, p2, alpha[:], x2, op0=OP.mult, op1=OP.add)
            # dots that only need r and s_{it-1}
            emit_dot(r_cur, r_cur, parts[:, 0:1])
            emit_dot(r_cur, s2, parts[:, 2:3])
            start_coil(1)

            for c in range(n_coil):
                Tt = [T(wk, [128, 512], f"T{i}", FPR) for i in range(2)]
                Zt = [T(wk, [128, 512], f"Z{i}", FPR) for i in range(2)]
                Ut = [T(wk, [128, 512], f"U{i}", FPR) for i in range(2)]
                ps1 = dft_stage(coils[c], True)
                emit_copies(ps1, Tt)
                ps2 = dft_stage(Tt, True)
                if c + 2 < n_coil:
                    start_coil(c + 2)
                emit_mask(ps2, Zt)
                ps3 = dft_stage(Zt, False)
                emit_copies(ps3, Ut)
                ps4 = dft_stage(Ut, False)
                emit_products(c, ps4, first=(c == 0))
                del coils[c]

            # mu_acc dots (two halves: slots 1 and 3; summed after the CC);
            # w = lam*r + acc early so it overlaps the collective
            emit_dot512(r_cur[:, 0:512], acc_re, parts[:, 1:2])
            emit_dot512(r_cur[:, 512:1024], acc_im, parts[:, 3:4])
            nc.vector.scalar_tensor_tensor(
                w2[:, 0:512], r_cur[:, 0:512], lam128[:], acc_re,
                op0=OP.mult, op1=OP.add)
            nc.vector.scalar_tensor_tensor(
                w2[:, 512:1024], r_cur[:, 512:1024], lam128[:], acc_im,
                op0=OP.mult, op1=OP.add)
            # pre-reduce the partial sums across partitions on the PE, then
            # one AllReduce; both bounce DMAs ride the idle sync HW queue
            psr = T(pq, [1, 8], "psr")
            nc.tensor.matmul(psr[0:1, 0:4], ones128[:, 0:1], parts[:, 0:4],
                             start=True, stop=True)
            small = T(st, [1, 8], f"small{it}")
            nc.scalar.copy(small[0:1, 0:4], psr[0:1, 0:4])
            nc.sync.dma_start(out=ccin[it][0:1, 0:4], in_=small[0:1, 0:4])
            if use_cc:
                nc.gpsimd.collective_compute(
                    "AllReduce", OP.add, replica_groups=group,
                    ins=[ccin[it][:]], outs=[ccout[it][:]])
                src_cc = ccout[it]
            else:
                src_cc = ccin[it]
            bc = T(st, [128, 8], f"bc{it}")
            nc.sync.dma_start(out=bc, in_=src_cc[:].to_broadcast([128, 8]))

            # scalar recurrences on [128,1] replicated values
            rho = bc[:, 0:1]
            nu = bc[:, 2:3]
            mu_acc = T(st, [128, 1], f"mua{it}")
            nc.vector.tensor_tensor(mu_acc, bc[:, 1:2], bc[:, 3:4], op=OP.add)
            mu = T(st, [128, 1], f"mu{it}")
            nc.vector.scalar_tensor_tensor(
                mu, rho, lam128[:], mu_acc, op0=OP.mult, op1=OP.add)
            if it == 0:
                beta = zero1
                dd = mu
            else:
                rro = T(st, [128, 1], f"rro{it}")
                nc.vector.reciprocal(rro, rho_old)
                beta = T(st, [128, 1], f"beta{it}")
                nc.vector.tensor_tensor(beta, rro, rho, op=OP.mult)
                # d = mu + beta*(2*nu + beta*d_old)
                q1 = T(st, [128, 1], f"q1{it}")
                nc.vector.tensor_tensor(q1, beta, d_old, op=OP.mult)
                q2 = T(st, [128, 1], f"q2{it}")
                nc.vector.scalar_tensor_tensor(
                    q2, nu, 2.0, q1, op0=OP.mult, op1=OP.add)
                q3 = T(st, [128, 1], f"q3{it}")
                nc.vector.tensor_tensor(q3, beta, q2, op=OP.mult)
                dd = T(st, [128, 1], f"dd{it}")
                nc.vector.tensor_tensor(dd, q3, mu, op=OP.add)
            rrd = T(st, [128, 1], f"rrd{it}")
            nc.vector.reciprocal(rrd, dd)
            alpha = T(st, [128, 1], f"al{it}")
            nc.vector.tensor_tensor(alpha, rrd, rho, op=OP.mult)
            nalpha = T(st, [128, 1], f"nal{it}")
            nc.vector.tensor_scalar_mul(nalpha, alpha, -1.0)
            nab = T(st, [128, 1], f"nab{it}")
            nc.vector.tensor_tensor(nab, nalpha, beta, op=OP.mult)
            rho_old = rho
            d_old = dd

            if it == n_cg - 1:
                # final p, x updates: p10 = r10 + beta10*p9 ; x11 += a10*p10
                nc.vector.scalar_tensor_tensor(
                    p2, p2, beta[:], r_cur, op0=OP.mult, op1=OP.add)
                nc.vector.scalar_tensor_tensor(
                    x2, p2, alpha[:], x2, op0=OP.mult, op1=OP.add)

        # ---- output: x tall-split -> natural [2, 256*256] ----
        for i in range(2):
            nc.sync.dma_start(
                out=dout[i].rearrange("(t p w) -> p t w", t=2, p=128, w=256),
                in_=x2[:, i * 512:(i + 1) * 512].rearrange(
                    "p (t w) -> p t w", t=2))

    if evsem:
        import bass_rust as _bass_rust
        _bass_rust.generate_event_semaphores(nc)
        mybir.codegen_inst_isa_subclasses(nc)
    return nc


# ------------------------------------------------------------------ runner

_CACHE = {}


def _get_nc(key=(N_CG, NCOIL, N_CORES, True)):
    if key not in _CACHE:
        _CACHE[key] = build_nc(*key)
    return _CACHE[key]


def make_in_maps(inputs):
    shared = _prep_shared(
        inputs["w1"], inputs["b1"], inputs["w2"], inputs["b2"], inputs["w3"],
        inputs["b3"], inputs["w4"], inputs["b4"], inputs["w5"], inputs["b5"],
        inputs["lam"])
    in_maps = []
    for b in range(N_CORES):
        m = dict(shared)
        m.update(_prep_core(
            np.asarray(inputs["atb"][b], np.float32),
            np.asarray(inputs["csm_real"][b], np.float32),
            np.asarray(inputs["csm_imag"][b], np.float32),
            np.asarray(inputs["mask"][b], np.float32)))
        in_maps.append(m)
    return in_maps


def run(inputs, trace=False, **kw):
    nc = _get_nc()
    in_maps = make_in_maps(inputs)
    res = run_bass_kernel_spmd(nc, in_maps, core_ids=list(range(N_CORES)),
                               trace=trace, **kw)
    out = np.stack([np.asarray(r["out"]).reshape(2, 256, 256)
                    for r in res.results]).astype(np.float32)
    return out, res


def kernel(**inputs):
    out, _ = run(inputs, trace=False)
    return out


# revision 17
# speedup vs baseline: 1.4016x; 1.1921x over previous
"""MoDL (CNN denoiser + CG data-consistency MRI recon) on 8 Trainium2 NeuronCores.

Sharding: data-parallel over batch B=8, one batch element per core. The CG
dot-products are global over the batch; the loop is restructured as
Chronopoulos-Gear CG (apply A to r, keep s=Ap via the recurrence
s = w + beta*s) so each iteration needs ONE tiny AllReduce placed right
after the matmul block instead of two serialized collectives, and every
vector update except "r" itself is off the critical path.

All FFTs are direct DFTs as PE matmuls (fp32r, full speed at free-dim>=256).
Each 1-D DFT stage uses the DATA as the stationary operand so every stage
output lands pre-transposed for the next stage (tall-split layout
[128, 2x256] throughout); re/im planes go to separate contiguous PSUM tiles
so every DVE op downstream is a fast contiguous [128,512] op (strided APs
and PSUM de-interleaving both measured much slower).

The coil sum (A r) accumulates per-coil into separate acc_re/acc_im SBUF
tiles (independent vector/gpsimd dependency chains, no strided
tensor_reduce), and emission is software-pipelined (form of coil c+2
interleaved with the mask/products of earlier coils) so the in-order engine
queues never head-of-line block the PE.

CNN: channels-on-partitions, shifted-window matmuls; 3x3 offsets packed in
pairs along the contraction axis. Layer-input strips live in three rotating
persistent SBUF tiles whose zero borders are maintained with thin row
memsets instead of a full-tile memset per strip.
"""

from contextlib import ExitStack

import numpy as np

import concourse.bass as bass
import concourse.tile as tile
from concourse import bacc
from concourse import mybir
from concourse.bass_utils import run_bass_kernel_spmd
from concourse import bass_isa

FP = mybir.dt.float32
FPR = mybir.dt.float32r
FH = mybir.dt.float16
AX = mybir.AxisListType
OP = mybir.AluOpType
AF = mybir.ActivationFunctionType

B, NCOIL, H, W = 8, 12, 256, 256
N_CG = 11
N_CORES = 8
HW = H * W


# ---------------------------------------------------------------- host prep

def _tall_split(plane):
    """[256,256] -> [128, 512]: col block t holds rows t*128..t*128+127."""
    return np.ascontiguousarray(
        plane.reshape(2, 128, 256).transpose(1, 0, 2).reshape(128, 512))


def _make_fmats():
    n = np.arange(256)
    Fm = np.exp(-2j * np.pi * np.outer(n, n) / 256) / 16.0
    fr = Fm.real.astype(np.float32)
    fi = Fm.imag.astype(np.float32)
    return np.stack([_tall_split(fr), _tall_split(fi), _tall_split(-fi)])


def _prep_shared(w1, b1, w2, b2, w3, b3, w4, b4, w5, b5, lam):
    out = {}
    out["fmats"] = _make_fmats()
    out["wpack1"] = np.ascontiguousarray(
        np.asarray(w1, np.float32).transpose(2, 3, 1, 0).reshape(18, 64))

    def pack(wl):
        wl = np.asarray(wl, np.float32)
        cout = wl.shape[0]
        g = np.zeros((6, 128, cout), np.float32)
        for idy in range(3):
            g[idy * 2 + 0, 0:64] = wl[:, :, idy, 0].T    # dx=-1
            g[idy * 2 + 0, 64:128] = wl[:, :, idy, 1].T  # dx=0
            g[idy * 2 + 1, 0:64] = wl[:, :, idy, 2].T    # dx=+1
        return g
    out["wpack2"] = pack(w2)
    out["wpack3"] = pack(w3)
    out["wpack4"] = pack(w4)
    out["wpack5"] = pack(w5)
    for i, bl in enumerate((b1, b2, b3, b4, b5)):
        out[f"bias{i + 1}"] = np.asarray(bl, np.float32).reshape(-1, 1)
    out["lam"] = np.asarray(lam, np.float32).reshape(1, 1)
    return out


def _prep_core(atb_b, csm_re_b, csm_im_b, mask_b):
    out = {}
    pad = np.zeros((2, 258, 258), np.float32)
    pad[:, 1:257, 1:257] = atb_b
    im = np.empty((18, 256, 256), np.float32)
    for idy in range(3):
        for idx in range(3):
            o = idy * 3 + idx
            im[2 * o:2 * o + 2] = pad[:, idy:idy + 256, idx:idx + 256]
    out["im2col"] = np.ascontiguousarray(im.reshape(18, HW))
    out["atb_ts"] = np.stack([_tall_split(atb_b[0]), _tall_split(atb_b[1])])
    out["csm_re"] = np.ascontiguousarray(
        np.stack([_tall_split(csm_re_b[c]) for c in range(NCOIL)], axis=1)
        .reshape(128, NCOIL * 512))
    out["csm_im"] = np.ascontiguousarray(
        np.stack([_tall_split(csm_im_b[c]) for c in range(NCOIL)], axis=1)
        .reshape(128, NCOIL * 512))
    out["mask_ts"] = _tall_split(mask_b)
    return out


# ------------------------------------------------------------- bass program

def build_nc(n_cg=N_CG, n_coil=NCOIL, n_cores=N_CORES, cnn=True, evsem=True,
             use_cc=True):
    _uid = [0]

    def T(pool, shape, tag, dt=FP):
        _uid[0] += 1
        return pool.tile(shape, dt, tag=tag, name=f"{tag}_{_uid[0]}")

    nc = bass.Bass(num_devices=n_cores)
    group = [list(range(n_cores))]

    din = {}
    for name, shape in [
        ("im2col", [18, HW]), ("atb_ts", [2, 128, 512]),
        ("csm_re", [128, n_coil * 512]), ("csm_im", [128, n_coil * 512]),
        ("mask_ts", [128, 512]), ("fmats", [3, 128, 512]),
        ("wpack1", [18, 64]), ("wpack2", [6, 128, 64]),
        ("wpack3", [6, 128, 64]), ("wpack4", [6, 128, 64]),
        ("wpack5", [6, 128, 2]),
        ("bias1", [64, 1]), ("bias2", [64, 1]), ("bias3", [64, 1]),
        ("bias4", [64, 1]), ("bias5", [2, 1]), ("lam", [1, 1]),
    ]:
        din[name] = nc.declare_dram_parameter(name, shape, FP, isOutput=False)
    dout = nc.declare_dram_parameter("out", [2, HW], FP, isOutput=True)

    acta = nc.dram_tensor("acta", [64, 256, 256], FP)
    actb = nc.dram_tensor("actb", [64, 256, 256], FP)
    h5d = nc.dram_tensor("h5d", [2, HW], FP)
    n_cc = n_cg
    ccin = [nc.dram_tensor(f"ccin{i}", [1, 8], FP) for i in range(n_cc)]
    cc_space = "Shared" if n_cores > 4 else "Local"
    ccout = [nc.dram_tensor(f"ccout{i}", [1, 8], FP, addr_space=cc_space)
             for i in range(n_cc)]

    with tile.TileContext(nc) as tc, ExitStack() as ctx:  # noqa: SIM117
        consts = ctx.enter_context(tc.tile_pool(name="consts", bufs=1))

        # ---- constants into SBUF (sync queue except casts; CNN weights are
        # first on the gpsimd queue so they gate nothing) ----
        fm = [T(consts, [128, 512], f"fm{j}", FH) for j in range(3)]
        for j in range(3):
            nc.gpsimd.dma_start(out=fm[j], in_=din["fmats"][j])
        csm_re = T(consts, [128, n_coil * 512], "csm_re", FH)
        csm_im = T(consts, [128, n_coil * 512], "csm_im", FH)
        nc.gpsimd.dma_start(out=csm_re, in_=din["csm_re"][:])
        nc.gpsimd.dma_start(out=csm_im, in_=din["csm_im"][:])
        mask = T(consts, [128, 512], "mask")
        nc.sync.dma_start(out=mask, in_=din["mask_ts"][:])
        lam128 = T(consts, [128, 1], "lam128")
        nc.sync.dma_start(out=lam128, in_=din["lam"][:].to_broadcast([128, 1]))
        ones128 = T(consts, [128, 1], "ones128")
        nc.vector.memset(ones128, 1.0)
        zrow = T(consts, [1, 8], "zrow")
        nc.vector.memset(zrow, 0.0)
        for i in range(n_cc):
            nc.sync.dma_start(out=ccin[i][:], in_=zrow)

        # =========================== CNN ===========================
        if cnn:
            with tc.tile_pool(name="cnnw", bufs=1) as cw, \
                 tc.tile_pool(name="cnnins", bufs=1) as cins, \
                 tc.tile_pool(name="cnnio", bufs=2) as cio, \
                 tc.tile_pool(name="cnnps", bufs=4, space="PSUM") as cps:
                wp1 = T(cw, [18, 64], "wp1", FPR)
                nc.gpsimd.dma_start(out=wp1, in_=din["wpack1"][:])
                bias = {}
                for l in (1, 2, 3, 4, 5):
                    cout = 2 if l == 5 else 64
                    bias[l] = T(cw, [cout, 1], f"b{l}")
                    nc.gpsimd.dma_start(out=bias[l], in_=din[f"bias{l}"][:])
                wp = {}
                for l in (2, 3, 4, 5):
                    cout = 2 if l == 5 else 64
                    wp[l] = [T(cw, [128, cout], f"w{l}g{g}", FPR)
                             for g in range(6)]
                    for g in range(6):
                        nc.gpsimd.dma_start(out=wp[l][g],
                                            in_=din[f"wpack{l}"][g])

                # ---- layer 1: K=18 im2col ----
                for s in range(16):
                    mv = T(cio, [18, 16 * 256], "l1mv", FPR)
                    nc.gpsimd.dma_start(
                        out=mv, in_=din["im2col"][:, s * 4096:(s + 1) * 4096])
                    ost = T(cio, [64, 16 * 256], "ostrip")
                    for k in range(8):
                        ps = T(cps, [64, 512], "ps")
                        nc.tensor.matmul(ps, wp1[:],
                                         mv[:, k * 512:(k + 1) * 512],
                                         start=True, stop=True)
                        nc.scalar.activation(ost[:, k * 512:(k + 1) * 512], ps,
                                             AF.Relu, bias=bias[1][:])
                    nc.sync.dma_start(
                        out=acta[:, s * 16:(s + 1) * 16, :],
                        in_=ost.rearrange("c (h w) -> c h w", w=256))

                # three rotating persistent input-strip tiles; zero borders
                # maintained with thin row memsets only
                instile = [T(cins, [128, 18 * 258], f"ins{i}", FPR)
                           for i in range(3)]
                ins3t = [t.rearrange("p (r w) -> p r w", r=18)
                         for t in instile]
                for i in range(3):
                    nc.vector.memset(instile[i].bitcast(mybir.dt.uint32), 0)

                # ---- layers 2..5 ----
                for l, (src, dst) in zip(
                        (2, 3, 4, 5), [(acta, actb), (actb, acta),
                                       (acta, actb), (actb, None)]):
                    cout = 2 if l == 5 else 64
                    if l > 2:
                        # strip 0 needs row 0 zero; dirty from prev layer
                        nc.vector.memset(
                            ins3t[0][:, 0:1, :].bitcast(mybir.dt.uint32), 0)
                    for s in range(16):
                        r0 = s * 16
                        ins3 = ins3t[s % 3]
                        ra = 0 if s > 0 else 1          # first valid ins row
                        rb = 18 if s < 15 else 17       # one past last valid
                        if s == 15:
                            # row 17 must be zero; dirty from strip 12
                            nc.vector.memset(
                                ins3[:, 17:18, :].bitcast(mybir.dt.uint32), 0)
                        nc.gpsimd.dma_start(
                            out=ins3[0:64, ra:rb, 1:257],
                            in_=src[:, r0 - 1 + ra:r0 - 1 + rb, :])
                        nc.gpsimd.dma_start(
                            out=ins3[64:128, ra:rb, 0:256],
                            in_=src[:, r0 - 1 + ra:r0 - 1 + rb, :])
                        ost = T(cio, [cout, 16 * 256], "ostrip")
                        for k in range(8):
                            ps = T(cps, [cout, 512], "ps")
                            for idy in range(3):
                                for half in range(2):
                                    g = idy * 2 + half
                                    dxo = 2 if half else 0
                                    mvap = ins3[:,
                                                k * 2 + idy:k * 2 + idy + 2,
                                                dxo:dxo + 256]
                                    nc.tensor.matmul(
                                        ps, wp[l][g], mvap,
                                        start=(g == 0), stop=(g == 5))
                            nc.scalar.activation(
                                ost[:, k * 512:(k + 1) * 512], ps,
                                AF.Relu if l != 5 else AF.Identity,
                                bias=bias[l][:])
                        if l == 5:
                            nc.sync.dma_start(
                                out=h5d[:, s * 4096:(s + 1) * 4096], in_=ost)
                        else:
                            nc.sync.dma_start(
                                out=dst[:, s * 16:(s + 1) * 16, :],
                                in_=ost.rearrange("c (h w) -> c h w", w=256))

        # =========================== CG ===========================
        st = ctx.enter_context(tc.tile_pool(name="cgstate", bufs=1))
        wk = ctx.enter_context(tc.tile_pool(name="cgwork", bufs=2))
        wkc = ctx.enter_context(tc.tile_pool(name="cgcoil", bufs=3))
        pp = ctx.enter_context(tc.tile_pool(name="cgps", bufs=3, space="PSUM"))
        pq = ctx.enter_context(tc.tile_pool(name="cgps2", bufs=1,
                                            space="PSUM"))

        # vectors: [128,1024] = [re tall-split | im tall-split]
        r_t = [T(st, [128, 1024], "ra"), T(st, [128, 1024], "rb")]
        p2 = T(st, [128, 1024], "p2")
        s2 = T(st, [128, 1024], "s2")
        x2 = T(st, [128, 1024], "x2")
        w2 = T(st, [128, 1024], "w2")
        acc_re = T(st, [128, 512], "acc_re", FH)
        acc_im = T(st, [128, 512], "acc_im", FH)
        r16 = T(st, [128, 1024], "r16", FH)
        lam1p = T(st, [128, 1], "lam1p")
        nc.vector.tensor_scalar_add(lam1p, lam128, 1.0)
        zero1 = T(st, [128, 1], "zero1")
        nc.vector.memset(zero1, 0.0)
        for t in (p2, s2, x2):
            nc.vector.memset(t, 0.0)

        # rhs = (1+lam)*atb + lam*h5 directly into r_t[0]
        for i in range(2):
            rsl = r_t[0][:, i * 512:(i + 1) * 512]
            nc.sync.dma_start(out=rsl, in_=din["atb_ts"][i])
            if cnn:
                h5p = T(wk, [128, 512], "ldh5")
                nc.sync.dma_start(
                    out=h5p,
                    in_=h5d[i].rearrange("(t p w) -> p t w",
                                         t=2, p=128, w=256))
                nc.vector.tensor_scalar(rsl, rsl, lam1p[:], None, op0=OP.mult)
                nc.vector.scalar_tensor_tensor(
                    rsl, h5p, lam128[:], rsl, op0=OP.mult, op1=OP.add)
            else:
                nc.vector.tensor_scalar(rsl, rsl, lam1p[:], None, op0=OP.mult)

        FWD = ((0, 2), (1, 0))   # re: Xr*Fr + Xi*(-Fi); im: Xr*Fi + Xi*Fr
        INV = ((0, 1), (2, 0))   # re: Xr*Fr + Xi*Fi;    im: Xr*(-Fi) + Xi*Fr

        def F(j, t):
            return fm[j][:, t * 256:(t + 1) * 256]

        def dft_stage(inp, fwd):
            """inp: 2 FPR planes [128,512]; out: 2 psum planes [128,512]."""
            combo = FWD if fwd else INV
            psA = T(pp, [128, 512], "psA")
            psB = T(pp, [128, 512], "psB")
            for m in range(2):
                for t in range(2):
                    for pl in range(2):
                        lt = inp[pl][:, t * 256 + m * 128:
                                     t * 256 + m * 128 + 128]
                        fst = (t == 0 and pl == 0)
                        lst = (t == 1 and pl == 1)
                        nc.tensor.matmul(psA[:, m * 256:(m + 1) * 256], lt,
                                         F(combo[0][pl], t),
                                         start=fst, stop=lst)
                        nc.tensor.matmul(psB[:, m * 256:(m + 1) * 256], lt,
                                         F(combo[1][pl], t),
                                         start=fst, stop=lst)
            return psA, psB

        def emit_form(c, coil, r_cur):
            """coil_re = cr*r_re - ci*r_im ; coil_im = cr*r_im + ci*r_re

            All fp16 on the vector engine (2x rate; gpsimd stays off these
            tiles to avoid the measured cross-engine SBUF contention)."""
            cr = csm_re[:, c * 512:(c + 1) * 512]
            cim = csm_im[:, c * 512:(c + 1) * 512]
            rre = r16[:, 0:512]
            rim = r16[:, 512:1024]
            m1 = T(wk, [128, 512], "fm1", FH)
            m2 = T(wk, [128, 512], "fm2", FH)
            m3 = T(wk, [128, 512], "fm3", FH)
            m4 = T(wk, [128, 512], "fm4", FH)
            nc.vector.tensor_tensor(m1, cr, rre, op=OP.mult)
            nc.vector.tensor_tensor(m2, cim, rim, op=OP.mult)
            nc.vector.tensor_tensor(m3, cr, rim, op=OP.mult)
            nc.vector.tensor_tensor(m4, cim, rre, op=OP.mult)
            nc.vector.tensor_tensor(coil[0], m1, m2, op=OP.subtract)
            nc.vector.tensor_tensor(coil[1], m3, m4, op=OP.add)

        def emit_mask(ps2, Zt):
            nc.vector.tensor_tensor(Zt[0], ps2[0], mask, op=OP.mult)
            nc.vector.tensor_tensor(Zt[1], ps2[1], mask, op=OP.mult)

        def emit_copies(ps1, Tt):
            nc.scalar.copy(Tt[0], ps1[0])
            nc.scalar.copy(Tt[1], ps1[1])

        def emit_products(c, ps4, first):
            """acc_re += cr*U_re + ci*U_im ; acc_im += cr*U_im - ci*U_re

            Product mults on vector (gpsimd cannot read PSUM); the re-chain
            accumulates on vector, the im-chain on gpsimd (separate acc
            tiles keep the chains independent)."""
            cr = csm_re[:, c * 512:(c + 1) * 512]
            cim = csm_im[:, c * 512:(c + 1) * 512]
            t1 = T(wk, [128, 512], "pt1", FH)
            t2 = T(wk, [128, 512], "pt2", FH)
            t3 = T(wk, [128, 512], "pt3", FH)
            t4 = T(wk, [128, 512], "pt4", FH)
            nc.vector.tensor_tensor(t1, cr, ps4[0], op=OP.mult)
            nc.vector.tensor_tensor(t2, cim, ps4[1], op=OP.mult)
            nc.vector.tensor_tensor(t3, cr, ps4[1], op=OP.mult)
            nc.vector.tensor_tensor(t4, cim, ps4[0], op=OP.mult)
            if first:
                nc.vector.tensor_tensor(acc_re, t1, t2, op=OP.add)
                nc.vector.tensor_tensor(acc_im, t3, t4, op=OP.subtract)
            else:
                nc.vector.tensor_tensor(acc_re, acc_re, t1, op=OP.add)
                nc.vector.tensor_tensor(acc_re, acc_re, t2, op=OP.add)
                nc.vector.tensor_tensor(acc_im, acc_im, t3, op=OP.add)
                nc.vector.tensor_tensor(acc_im, acc_im, t4, op=OP.subtract)

        def emit_dot(a, b, part_slot):
            scrap = T(wk, [128, 1024], "dsc")
            nc.vector.scalar_tensor_tensor(
                scrap, a, 1.0, b, op0=OP.mult, op1=OP.mult,
                accum_out=part_slot)

        def emit_dot512(a, b, part_slot):
            scrap = T(wk, [128, 512], "dsc5")
            nc.vector.scalar_tensor_tensor(
                scrap, a, 1.0, b, op0=OP.mult, op1=OP.mult,
                accum_out=part_slot)

        # ---------------- CG loop (Chronopoulos-Gear, unrolled) --------
        rho_old = None
        d_old = None
        alpha = None
        nalpha = None
        nab = None
        beta = None
        for it in range(n_cg):
            r_prev = r_t[(it + 1) % 2]
            r_cur = r_t[it % 2]
            parts = T(st, [128, 8], f"parts{it}")
            if it > 0:
                # r <- r_prev - alpha*(beta*s_old + w)  [s recurrence fused]
                rtmp = T(wk, [128, 1024], "rt")
                nc.vector.scalar_tensor_tensor(
                    rtmp, s2, nab[:], r_prev, op0=OP.mult, op1=OP.add)
                nc.vector.scalar_tensor_tensor(
                    r_cur, w2, nalpha[:], rtmp, op0=OP.mult, op1=OP.add)

            coils = {}

            def start_coil(c, _r=r_cur):
                coil = [T(wkc, [128, 512], f"coil{i}", FH) for i in range(2)]
                emit_form(c, coil, _r)
                coils[c] = coil

            # fp16 shadow of r for the coil pipeline (scalar engine)
            nc.scalar.copy(r16, r_cur)
            start_coil(0)
            if it > 0:
                # deferred recurrences (off the critical path):
                # s <- w + beta*s ; p <- r_prev + beta*p ; x <- x + alpha*p
                nc.vector.scalar_tensor_tensor(
                    s2, s2, beta[:], w2, op0=OP.mult, op1=OP.add)
                nc.vector.scalar_tensor_tensor(
                    p2, p2, beta[:], r_prev, op0=OP.mult, op1=OP.add)
                nc.vector.scalar_tensor_tensor(
                    x2, p2, alpha[:], x2, op0=OP.mult, op1=OP.add)
            # dots that only need r and s_{it-1}
            emit_dot(r_cur, r_cur, parts[:, 0:1])
            emit_dot(r_cur, s2, parts[:, 2:3])
            start_coil(1)

            for c in range(n_coil):
                Tt = [T(wk, [128, 512], f"T{i}", FH) for i in range(2)]
                Zt = [T(wk, [128, 512], f"Z{i}", FH) for i in range(2)]
                Ut = [T(wk, [128, 512], f"U{i}", FH) for i in range(2)]
                ps1 = dft_stage(coils[c], True)
                emit_copies(ps1, Tt)
                ps2 = dft_stage(Tt, True)
                if c + 2 < n_coil:
                    start_coil(c + 2)
                emit_mask(ps2, Zt)
                ps3 = dft_stage(Zt, False)
                emit_copies(ps3, Ut)
                ps4 = dft_stage(Ut, False)
                emit_products(c, ps4, first=(c == 0))
                del coils[c]

            # mu_acc dots (two halves: slots 1 and 3; summed after the CC);
            # w = lam*r + acc early so it overlaps the collective
            emit_dot512(r_cur[:, 0:512], acc_re, parts[:, 1:2])
            emit_dot512(r_cur[:, 512:1024], acc_im, parts[:, 3:4])
            nc.vector.scalar_tensor_tensor(
                w2[:, 0:512], r_cur[:, 0:512], lam128[:], acc_re,
                op0=OP.mult, op1=OP.add)
            nc.vector.scalar_tensor_tensor(
                w2[:, 512:1024], r_cur[:, 512:1024], lam128[:], acc_im,
                op0=OP.mult, op1=OP.add)
            # pre-reduce the partial sums across partitions on the PE, then
            # one AllReduce; both bounce DMAs ride the idle sync HW queue
            psr = T(pq, [1, 8], "psr")
            nc.tensor.matmul(psr[0:1, 0:4], ones128[:, 0:1], parts[:, 0:4],
                             start=True, stop=True)
            small = T(st, [1, 8], f"small{it}")
            nc.scalar.copy(small[0:1, 0:4], psr[0:1, 0:4])
            nc.sync.dma_start(out=ccin[it][0:1, 0:4], in_=small[0:1, 0:4])
            if use_cc:
                nc.gpsimd.collective_compute(
                    "AllReduce", OP.add, replica_groups=group,
                    ins=[ccin[it][:]], outs=[ccout[it][:]])
                src_cc = ccout[it]
            else:
                src_cc = ccin[it]
            bc = T(st, [128, 8], f"bc{it}")
            nc.sync.dma_start(out=bc, in_=src_cc[:].to_broadcast([128, 8]))

            # scalar recurrences on [128,1] replicated values
            rho = bc[:, 0:1]
            nu = bc[:, 2:3]
            mu_acc = T(st, [128, 1], f"mua{it}")
            nc.vector.tensor_tensor(mu_acc, bc[:, 1:2], bc[:, 3:4], op=OP.add)
            mu = T(st, [128, 1], f"mu{it}")
            nc.vector.scalar_tensor_tensor(
                mu, rho, lam128[:], mu_acc, op0=OP.mult, op1=OP.add)
            if it == 0:
                beta = zero1
                dd = mu
            else:
                rro = T(st, [128, 1], f"rro{it}")
                nc.vector.reciprocal(rro, rho_old)
                beta = T(st, [128, 1], f"beta{it}")
                nc.vector.tensor_tensor(beta, rro, rho, op=OP.mult)
                # d = mu + beta*(2*nu + beta*d_old)
                q1 = T(st, [128, 1], f"q1{it}")
                nc.vector.tensor_tensor(q1, beta, d_old, op=OP.mult)
                q2 = T(st, [128, 1], f"q2{it}")
                nc.vector.scalar_tensor_tensor(
                    q2, nu, 2.0, q1, op0=OP.mult, op1=OP.add)
                q3 = T(st, [128, 1], f"q3{it}")
                nc.vector.tensor_tensor(q3, beta, q2, op=OP.mult)
                dd = T(st, [128, 1], f"dd{it}")
                nc.vector.tensor_tensor(dd, q3, mu, op=OP.add)
            rrd = T(st, [128, 1], f"rrd{it}")
            nc.vector.reciprocal(rrd, dd)
            alpha = T(st, [128, 1], f"al{it}")
            nc.vector.tensor_tensor(alpha, rrd, rho, op=OP.mult)
            nalpha = T(st, [128, 1], f"nal{it}")
            nc.vector.tensor_scalar_mul(nalpha, alpha, -1.0)
            nab = T(st, [128, 1], f"nab{it}")
            nc.vector.tensor_tensor(nab, nalpha, beta, op=OP.mult)
            rho_old = rho
            d_old = dd

            if it == n_cg - 1:
                # final p, x updates: p10 = r10 + beta10*p9 ; x11 += a10*p10
                nc.vector.scalar_tensor_tensor(
                    p2, p2, beta[:], r_cur, op0=OP.mult, op1=OP.add)
                nc.vector.scalar_tensor_tensor(
                    x2, p2, alpha[:], x2, op0=OP.mult, op1=OP.add)

        # ---- output: x tall-split -> natural [2, 256*256] ----
        for i in range(2):
            nc.sync.dma_start(
                out=dout[i].rearrange("(t p w) -> p t w", t=2, p=128, w=256),
                in_=x2[:, i * 512:(i + 1) * 512].rearrange(
                    "p (t w) -> p t w", t=2))

    if evsem:
        import bass_rust as _bass_rust
        _bass_rust.generate_event_semaphores(nc)
        mybir.codegen_inst_isa_subclasses(nc)
    return nc


# ------------------------------------------------------------------ runner

_CACHE = {}


def _get_nc(key=(N_CG, NCOIL, N_CORES, True)):
    if key not in _CACHE:
        _CACHE[key] = build_nc(*key)
    return _CACHE[key]


def make_in_maps(inputs):
    shared = _prep_shared(
        inputs["w1"], inputs["b1"], inputs["w2"], inputs["b2"], inputs["w3"],
        inputs["b3"], inputs["w4"], inputs["b4"], inputs["w5"], inputs["b5"],
        inputs["lam"])
    in_maps = []
    for b in range(N_CORES):
        m = dict(shared)
        m.update(_prep_core(
            np.asarray(inputs["atb"][b], np.float32),
            np.asarray(inputs["csm_real"][b], np.float32),
            np.asarray(inputs["csm_imag"][b], np.float32),
            np.asarray(inputs["mask"][b], np.float32)))
        in_maps.append(m)
    return in_maps


def run(inputs, trace=False, **kw):
    nc = _get_nc()
    in_maps = make_in_maps(inputs)
    res = run_bass_kernel_spmd(nc, in_maps, core_ids=list(range(N_CORES)),
                               trace=trace, **kw)
    out = np.stack([np.asarray(r["out"]).reshape(2, 256, 256)
                    for r in res.results]).astype(np.float32)
    return out, res


def kernel(**inputs):
    out, _ = run(inputs, trace=False)
    return out


# revision 24
# speedup vs baseline: 1.5384x; 1.0976x over previous
"""MoDL (CNN denoiser + CG data-consistency MRI recon) on 8 Trainium2 NeuronCores.

Sharding: data-parallel over batch B=8, one batch element per core. The CG
dot-products are global over the batch; the loop is restructured as
Chronopoulos-Gear CG (apply A to r, keep s=Ap via the recurrence
s = w + beta*s) so each iteration needs ONE tiny AllReduce placed right
after the matmul block instead of two serialized collectives, and every
vector update except "r" itself is off the critical path.

All FFTs are direct DFTs as PE matmuls (fp32r, full speed at free-dim>=256).
Each 1-D DFT stage uses the DATA as the stationary operand so every stage
output lands pre-transposed for the next stage (tall-split layout
[128, 2x256] throughout); re/im planes go to separate contiguous PSUM tiles
so every DVE op downstream is a fast contiguous [128,512] op (strided APs
and PSUM de-interleaving both measured much slower).

The coil sum (A r) accumulates per-coil into separate acc_re/acc_im SBUF
tiles (independent vector/gpsimd dependency chains, no strided
tensor_reduce), and emission is software-pipelined (form of coil c+2
interleaved with the mask/products of earlier coils) so the in-order engine
queues never head-of-line block the PE.

CNN: channels-on-partitions, shifted-window matmuls; 3x3 offsets packed in
pairs along the contraction axis. Layer-input strips live in three rotating
persistent SBUF tiles whose zero borders are maintained with thin row
memsets instead of a full-tile memset per strip.
"""

from contextlib import ExitStack

import numpy as np

import concourse.bass as bass
import concourse.tile as tile
from concourse import bacc
from concourse import mybir
from concourse.bass_utils import run_bass_kernel_spmd
from concourse import bass_isa

FP = mybir.dt.float32
FPR = mybir.dt.float32r
FH = mybir.dt.float16
AX = mybir.AxisListType
OP = mybir.AluOpType
AF = mybir.ActivationFunctionType

B, NCOIL, H, W = 8, 12, 256, 256
N_CG = 11
N_CORES = 8
HW = H * W


# ---------------------------------------------------------------- host prep

def _tall_split(plane):
    """[256,256] -> [128, 512]: col block t holds rows t*128..t*128+127."""
    return np.ascontiguousarray(
        plane.reshape(2, 128, 256).transpose(1, 0, 2).reshape(128, 512))


def _make_fmats():
    n = np.arange(256)
    Fm = np.exp(-2j * np.pi * np.outer(n, n) / 256) / 16.0
    fr = Fm.real.astype(np.float32)
    fi = Fm.imag.astype(np.float32)
    return np.stack([_tall_split(fr), _tall_split(fi), _tall_split(-fi)])


def _prep_shared(w1, b1, w2, b2, w3, b3, w4, b4, w5, b5, lam):
    out = {}
    out["fmats"] = _make_fmats()
    out["wpack1"] = np.ascontiguousarray(
        np.asarray(w1, np.float32).transpose(2, 3, 1, 0).reshape(18, 64))

    def pack(wl):
        wl = np.asarray(wl, np.float32)
        cout = wl.shape[0]
        g = np.zeros((6, 128, cout), np.float32)
        for idy in range(3):
            g[idy * 2 + 0, 0:64] = wl[:, :, idy, 0].T    # dx=-1
            g[idy * 2 + 0, 64:128] = wl[:, :, idy, 1].T  # dx=0
            g[idy * 2 + 1, 0:64] = wl[:, :, idy, 2].T    # dx=+1
        return g
    out["wpack2"] = pack(w2)
    out["wpack3"] = pack(w3)
    out["wpack4"] = pack(w4)
    out["wpack5"] = pack(w5)
    for i, bl in enumerate((b1, b2, b3, b4, b5)):
        out[f"bias{i + 1}"] = np.asarray(bl, np.float32).reshape(-1, 1)
    out["lam"] = np.asarray(lam, np.float32).reshape(1, 1)
    return out


def _prep_core(atb_b, csm_re_b, csm_im_b, mask_b):
    out = {}
    pad = np.zeros((2, 258, 258), np.float32)
    pad[:, 1:257, 1:257] = atb_b
    im = np.empty((18, 256, 256), np.float32)
    for idy in range(3):
        for idx in range(3):
            o = idy * 3 + idx
            im[2 * o:2 * o + 2] = pad[:, idy:idy + 256, idx:idx + 256]
    out["im2col"] = np.ascontiguousarray(im.reshape(18, HW)).astype(np.float16)
    out["atb_ts"] = np.stack([_tall_split(atb_b[0]), _tall_split(atb_b[1])])
    out["csm_re"] = np.ascontiguousarray(
        np.stack([_tall_split(csm_re_b[c]) for c in range(NCOIL)], axis=1)
        .reshape(128, NCOIL * 512))
    out["csm_im"] = np.ascontiguousarray(
        np.stack([_tall_split(csm_im_b[c]) for c in range(NCOIL)], axis=1)
        .reshape(128, NCOIL * 512))
    out["mask_ts"] = _tall_split(mask_b)
    return out


# ------------------------------------------------------------- bass program

def build_nc(n_cg=N_CG, n_coil=NCOIL, n_cores=N_CORES, cnn=True, evsem=True,
             use_cc=True):
    _uid = [0]

    def T(pool, shape, tag, dt=FP):
        _uid[0] += 1
        return pool.tile(shape, dt, tag=tag, name=f"{tag}_{_uid[0]}")

    nc = bass.Bass(num_devices=n_cores)
    group = [list(range(n_cores))]

    din = {}
    for name, shape in [
        ("atb_ts", [2, 128, 512]),
        ("csm_re", [128, n_coil * 512]), ("csm_im", [128, n_coil * 512]),
        ("mask_ts", [128, 512]), ("fmats", [3, 128, 512]),
        ("wpack1", [18, 64]), ("wpack2", [6, 128, 64]),
        ("wpack3", [6, 128, 64]), ("wpack4", [6, 128, 64]),
        ("wpack5", [6, 128, 2]),
        ("bias1", [64, 1]), ("bias2", [64, 1]), ("bias3", [64, 1]),
        ("bias4", [64, 1]), ("bias5", [2, 1]), ("lam", [1, 1]),
    ]:
        din[name] = nc.declare_dram_parameter(name, shape, FP, isOutput=False)
    din["im2col"] = nc.declare_dram_parameter("im2col", [18, HW], FH,
                                              isOutput=False)
    dout = nc.declare_dram_parameter("out", [2, HW], FP, isOutput=True)

    acta = nc.dram_tensor("acta", [64, 256, 256], FH)
    actb = nc.dram_tensor("actb", [64, 256, 256], FH)
    h5d = nc.dram_tensor("h5d", [2, HW], FH)
    n_cc = n_cg
    ccin = [nc.dram_tensor(f"ccin{i}", [1, 8], FP) for i in range(n_cc)]
    cc_space = "Shared" if n_cores > 4 else "Local"
    ccout = [nc.dram_tensor(f"ccout{i}", [1, 8], FP, addr_space=cc_space)
             for i in range(n_cc)]

    with tile.TileContext(nc) as tc, ExitStack() as ctx:  # noqa: SIM117
        consts = ctx.enter_context(tc.tile_pool(name="consts", bufs=1))

        # ---- constant tiles; the big fp16 cast-DMAs (fm, csm) are emitted
        # AFTER the CNN layer-1 loop so they don't delay the first im2col
        # transfers on the gpsimd queue ----
        fm = [T(consts, [128, 512], f"fm{j}", FH) for j in range(3)]
        csm_re = T(consts, [128, n_coil * 512], "csm_re", FH)
        csm_im = T(consts, [128, n_coil * 512], "csm_im", FH)

        def load_cg_consts():
            for j in range(3):
                nc.gpsimd.dma_start(out=fm[j], in_=din["fmats"][j])
            nc.gpsimd.dma_start(out=csm_re, in_=din["csm_re"][:])
            nc.gpsimd.dma_start(out=csm_im, in_=din["csm_im"][:])

        mask = T(consts, [128, 512], "mask")
        nc.sync.dma_start(out=mask, in_=din["mask_ts"][:])
        lam128 = T(consts, [128, 1], "lam128")
        nc.sync.dma_start(out=lam128, in_=din["lam"][:].to_broadcast([128, 1]))
        ones128 = T(consts, [128, 1], "ones128")
        nc.vector.memset(ones128, 1.0)
        zrow = T(consts, [1, 8], "zrow")
        nc.vector.memset(zrow, 0.0)
        for i in range(n_cc):
            nc.sync.dma_start(out=ccin[i][:], in_=zrow)

        # =========================== CNN ===========================
        if cnn:
            with tc.tile_pool(name="cnnw", bufs=1) as cw, \
                 tc.tile_pool(name="cnnins", bufs=1) as cins, \
                 tc.tile_pool(name="cnnio", bufs=2) as cio, \
                 tc.tile_pool(name="cnnps", bufs=4, space="PSUM") as cps:
                wp1 = T(cw, [18, 64], "wp1", FH)
                nc.gpsimd.dma_start(out=wp1, in_=din["wpack1"][:])
                bias = {}
                for l in (1, 2, 3, 4, 5):
                    cout = 2 if l == 5 else 64
                    bias[l] = T(cw, [cout, 1], f"b{l}")
                    nc.gpsimd.dma_start(out=bias[l], in_=din[f"bias{l}"][:])
                wp = {}
                for l in (2, 3, 4, 5):
                    cout = 2 if l == 5 else 64
                    wp[l] = [T(cw, [128, cout], f"w{l}g{g}", FH)
                             for g in range(6)]
                    for g in range(6):
                        nc.gpsimd.dma_start(out=wp[l][g],
                                            in_=din[f"wpack{l}"][g])

                # ---- layer 1: K=18 im2col ----
                for s in range(16):
                    mv = T(cio, [18, 16 * 256], "l1mv", FH)
                    nc.gpsimd.dma_start(
                        out=mv, in_=din["im2col"][:, s * 4096:(s + 1) * 4096])
                    ost = T(cio, [64, 16 * 256], "ostrip", FH)
                    for k in range(8):
                        ps = T(cps, [64, 512], "ps")
                        nc.tensor.matmul(ps, wp1[:],
                                         mv[:, k * 512:(k + 1) * 512],
                                         start=True, stop=True)
                        nc.scalar.activation(ost[:, k * 512:(k + 1) * 512], ps,
                                             AF.Relu, bias=bias[1][:])
                    nc.sync.dma_start(
                        out=acta[:, s * 16:(s + 1) * 16, :],
                        in_=ost.rearrange("c (h w) -> c h w", w=256))

                # big CG const casts ride the gpsimd queue from here on
                load_cg_consts()

                # three rotating persistent input-strip tiles; zero borders
                # maintained with thin row memsets only
                instile = [T(cins, [128, 18 * 258], f"ins{i}", FH)
                           for i in range(3)]
                ins3t = [t.rearrange("p (r w) -> p r w", r=18)
                         for t in instile]
                for i in range(3):
                    nc.vector.memset(instile[i].bitcast(mybir.dt.uint32), 0)

                # ---- layers 2..5 ----
                for l, (src, dst) in zip(
                        (2, 3, 4, 5), [(acta, actb), (actb, acta),
                                       (acta, actb), (actb, None)]):
                    cout = 2 if l == 5 else 64
                    if l > 2:
                        # strip 0 needs row 0 zero; dirty from prev layer
                        nc.vector.memset(
                            ins3t[0][:, 0:1, :].bitcast(mybir.dt.uint32), 0)
                    for s in range(16):
                        r0 = s * 16
                        ins3 = ins3t[s % 3]
                        ra = 0 if s > 0 else 1          # first valid ins row
                        rb = 18 if s < 15 else 17       # one past last valid
                        if s == 15:
                            # row 17 must be zero; dirty from strip 12
                            nc.vector.memset(
                                ins3[:, 17:18, :].bitcast(mybir.dt.uint32), 0)
                        nc.gpsimd.dma_start(
                            out=ins3[0:64, ra:rb, 1:257],
                            in_=src[:, r0 - 1 + ra:r0 - 1 + rb, :])
                        nc.gpsimd.dma_start(
                            out=ins3[64:128, ra:rb, 0:256],
                            in_=src[:, r0 - 1 + ra:r0 - 1 + rb, :])
                        ost = T(cio, [cout, 16 * 256], "ostrip", FH)
                        for k in range(8):
                            ps = T(cps, [cout, 512], "ps")
                            for idy in range(3):
                                for half in range(2):
                                    g = idy * 2 + half
                                    dxo = 2 if half else 0
                                    mvap = ins3[:,
                                                k * 2 + idy:k * 2 + idy + 2,
                                                dxo:dxo + 256]
                                    nc.tensor.matmul(
                                        ps, wp[l][g], mvap,
                                        start=(g == 0), stop=(g == 5))
                            nc.scalar.activation(
                                ost[:, k * 512:(k + 1) * 512], ps,
                                AF.Relu if l != 5 else AF.Identity,
                                bias=bias[l][:])
                        if l == 5:
                            nc.sync.dma_start(
                                out=h5d[:, s * 4096:(s + 1) * 4096], in_=ost)
                        else:
                            nc.sync.dma_start(
                                out=dst[:, s * 16:(s + 1) * 16, :],
                                in_=ost.rearrange("c (h w) -> c h w", w=256))

        # =========================== CG ===========================
        if not cnn:
            load_cg_consts()
        st = ctx.enter_context(tc.tile_pool(name="cgstate", bufs=1))
        wk = ctx.enter_context(tc.tile_pool(name="cgwork", bufs=2))
        wkc = ctx.enter_context(tc.tile_pool(name="cgcoil", bufs=3))
        pp = ctx.enter_context(tc.tile_pool(name="cgps", bufs=3, space="PSUM"))
        pq = ctx.enter_context(tc.tile_pool(name="cgps2", bufs=1,
                                            space="PSUM"))

        # vectors: [128,1024] = [re tall-split | im tall-split]
        r_t = [T(st, [128, 1024], "ra"), T(st, [128, 1024], "rb")]
        p2 = T(st, [128, 1024], "p2")
        s2 = T(st, [128, 1024], "s2")
        x2 = T(st, [128, 1024], "x2")
        w2 = T(st, [128, 1024], "w2")
        acc_re = T(st, [128, 512], "acc_re", FH)
        acc_im = T(st, [128, 512], "acc_im", FH)
        r16 = T(st, [128, 1024], "r16", FH)
        lam1p = T(st, [128, 1], "lam1p")
        nc.vector.tensor_scalar_add(lam1p, lam128, 1.0)
        zero1 = T(st, [128, 1], "zero1")
        nc.vector.memset(zero1, 0.0)
        for t in (p2, s2, x2):
            nc.vector.memset(t, 0.0)

        # rhs = (1+lam)*atb + lam*h5 directly into r_t[0]
        for i in range(2):
            rsl = r_t[0][:, i * 512:(i + 1) * 512]
            nc.sync.dma_start(out=rsl, in_=din["atb_ts"][i])
            if cnn:
                h5p = T(wk, [128, 512], "ldh5", FH)
                nc.sync.dma_start(
                    out=h5p,
                    in_=h5d[i].rearrange("(t p w) -> p t w",
                                         t=2, p=128, w=256))
                nc.vector.tensor_scalar(rsl, rsl, lam1p[:], None, op0=OP.mult)
                nc.vector.scalar_tensor_tensor(
                    rsl, h5p, lam128[:], rsl, op0=OP.mult, op1=OP.add)
            else:
                nc.vector.tensor_scalar(rsl, rsl, lam1p[:], None, op0=OP.mult)

        FWD = ((0, 2), (1, 0))   # re: Xr*Fr + Xi*(-Fi); im: Xr*Fi + Xi*Fr
        INV = ((0, 1), (2, 0))   # re: Xr*Fr + Xi*Fi;    im: Xr*(-Fi) + Xi*Fr

        def F(j, t):
            return fm[j][:, t * 256:(t + 1) * 256]

        def dft_stage(inp, fwd):
            """inp: 2 FPR planes [128,512]; out: 2 psum planes [128,512]."""
            combo = FWD if fwd else INV
            psA = T(pp, [128, 512], "psA")
            psB = T(pp, [128, 512], "psB")
            for m in range(2):
                for t in range(2):
                    for pl in range(2):
                        lt = inp[pl][:, t * 256 + m * 128:
                                     t * 256 + m * 128 + 128]
                        fst = (t == 0 and pl == 0)
                        lst = (t == 1 and pl == 1)
                        nc.tensor.matmul(psA[:, m * 256:(m + 1) * 256], lt,
                                         F(combo[0][pl], t),
                                         start=fst, stop=lst)
                        nc.tensor.matmul(psB[:, m * 256:(m + 1) * 256], lt,
                                         F(combo[1][pl], t),
                                         start=fst, stop=lst)
            return psA, psB

        def emit_form(c, coil, r_cur):
            """coil_re = cr*r_re - ci*r_im ; coil_im = cr*r_im + ci*r_re

            All fp16 on the vector engine (2x rate; gpsimd stays off these
            tiles to avoid the measured cross-engine SBUF contention)."""
            cr = csm_re[:, c * 512:(c + 1) * 512]
            cim = csm_im[:, c * 512:(c + 1) * 512]
            rre = r16[:, 0:512]
            rim = r16[:, 512:1024]
            m1 = T(wk, [128, 512], "fm1", FH)
            m2 = T(wk, [128, 512], "fm2", FH)
            m3 = T(wk, [128, 512], "fm3", FH)
            m4 = T(wk, [128, 512], "fm4", FH)
            nc.vector.tensor_tensor(m1, cr, rre, op=OP.mult)
            nc.vector.tensor_tensor(m2, cim, rim, op=OP.mult)
            nc.vector.tensor_tensor(m3, cr, rim, op=OP.mult)
            nc.vector.tensor_tensor(m4, cim, rre, op=OP.mult)
            nc.vector.tensor_tensor(coil[0], m1, m2, op=OP.subtract)
            nc.vector.tensor_tensor(coil[1], m3, m4, op=OP.add)

        def emit_mask(ps2, Zt):
            nc.vector.tensor_tensor(Zt[0], ps2[0], mask, op=OP.mult)
            nc.vector.tensor_tensor(Zt[1], ps2[1], mask, op=OP.mult)

        def emit_copies(ps1, Tt):
            nc.scalar.copy(Tt[0], ps1[0])
            nc.scalar.copy(Tt[1], ps1[1])

        def emit_products(c, ps4, first):
            """acc_re += cr*U_re + ci*U_im ; acc_im += cr*U_im - ci*U_re

            Product mults on vector (gpsimd cannot read PSUM); the re-chain
            accumulates on vector, the im-chain on gpsimd (separate acc
            tiles keep the chains independent)."""
            cr = csm_re[:, c * 512:(c + 1) * 512]
            cim = csm_im[:, c * 512:(c + 1) * 512]
            t1 = T(wk, [128, 512], "pt1", FH)
            t2 = T(wk, [128, 512], "pt2", FH)
            t3 = T(wk, [128, 512], "pt3", FH)
            t4 = T(wk, [128, 512], "pt4", FH)
            nc.vector.tensor_tensor(t1, cr, ps4[0], op=OP.mult)
            nc.vector.tensor_tensor(t2, cim, ps4[1], op=OP.mult)
            nc.vector.tensor_tensor(t3, cr, ps4[1], op=OP.mult)
            nc.vector.tensor_tensor(t4, cim, ps4[0], op=OP.mult)
            if first:
                nc.vector.tensor_tensor(acc_re, t1, t2, op=OP.add)
                nc.vector.tensor_tensor(acc_im, t3, t4, op=OP.subtract)
            else:
                nc.vector.tensor_tensor(acc_re, acc_re, t1, op=OP.add)
                nc.vector.tensor_tensor(acc_re, acc_re, t2, op=OP.add)
                nc.vector.tensor_tensor(acc_im, acc_im, t3, op=OP.add)
                nc.vector.tensor_tensor(acc_im, acc_im, t4, op=OP.subtract)

        def emit_dot(a, b, part_slot):
            scrap = T(wk, [128, 1024], "dsc")
            nc.vector.scalar_tensor_tensor(
                scrap, a, 1.0, b, op0=OP.mult, op1=OP.mult,
                accum_out=part_slot)

        def emit_dot512(a, b, part_slot):
            scrap = T(wk, [128, 512], "dsc5")
            nc.vector.scalar_tensor_tensor(
                scrap, a, 1.0, b, op0=OP.mult, op1=OP.mult,
                accum_out=part_slot)

        # ---------------- CG loop (Chronopoulos-Gear, unrolled) --------
        rho_old = None
        d_old = None
        alpha = None
        nalpha = None
        nab = None
        beta = None
        for it in range(n_cg):
            r_prev = r_t[(it + 1) % 2]
            r_cur = r_t[it % 2]
            parts = T(st, [128, 8], f"parts{it}")
            if it > 0:
                # r <- r_prev - alpha*(beta*s_old + w)  [s recurrence fused]
                rtmp = T(wk, [128, 1024], "rt")
                nc.vector.scalar_tensor_tensor(
                    rtmp, s2, nab[:], r_prev, op0=OP.mult, op1=OP.add)
                nc.vector.scalar_tensor_tensor(
                    r_cur, w2, nalpha[:], rtmp, op0=OP.mult, op1=OP.add)

            coils = {}

            def start_coil(c, _r=r_cur):
                coil = [T(wkc, [128, 512], f"coil{i}", FH) for i in range(2)]
                emit_form(c, coil, _r)
                coils[c] = coil

            # fp16 shadow of r for the coil pipeline (scalar engine)
            nc.scalar.copy(r16, r_cur)
            start_coil(0)
            if it > 0:
                # deferred recurrences (off the critical path):
                # s <- w + beta*s ; p <- r_prev + beta*p ; x <- x + alpha*p
                nc.vector.scalar_tensor_tensor(
                    s2, s2, beta[:], w2, op0=OP.mult, op1=OP.add)
                nc.vector.scalar_tensor_tensor(
                    p2, p2, beta[:], r_prev, op0=OP.mult, op1=OP.add)
                nc.vector.scalar_tensor_tensor(
                    x2, p2, alpha[:], x2, op0=OP.mult, op1=OP.add)
            # dots that only need r and s_{it-1}
            emit_dot(r_cur, r_cur, parts[:, 0:1])
            emit_dot(r_cur, s2, parts[:, 2:3])
            start_coil(1)

            for c in range(n_coil):
                Tt = [T(wk, [128, 512], f"T{i}", FH) for i in range(2)]
                Zt = [T(wk, [128, 512], f"Z{i}", FH) for i in range(2)]
                Ut = [T(wk, [128, 512], f"U{i}", FH) for i in range(2)]
                ps1 = dft_stage(coils[c], True)
                emit_copies(ps1, Tt)
                ps2 = dft_stage(Tt, True)
                if c + 2 < n_coil:
                    start_coil(c + 2)
                emit_mask(ps2, Zt)
                ps3 = dft_stage(Zt, False)
                emit_copies(ps3, Ut)
                ps4 = dft_stage(Ut, False)
                emit_products(c, ps4, first=(c == 0))
                del coils[c]

            # mu_acc dots (two halves: slots 1 and 3; summed after the CC);
            # w = lam*r + acc early so it overlaps the collective
            emit_dot512(r_cur[:, 0:512], acc_re, parts[:, 1:2])
            emit_dot512(r_cur[:, 512:1024], acc_im, parts[:, 3:4])
            nc.vector.scalar_tensor_tensor(
                w2[:, 0:512], r_cur[:, 0:512], lam128[:], acc_re,
                op0=OP.mult, op1=OP.add)
            nc.vector.scalar_tensor_tensor(
                w2[:, 512:1024], r_cur[:, 512:1024], lam128[:], acc_im,
                op0=OP.mult, op1=OP.add)
            # pre-reduce the partial sums across partitions on the PE, then
            # one AllReduce; both bounce DMAs ride the idle sync HW queue
            psr = T(pq, [1, 8], "psr")
            nc.tensor.matmul(psr[0:1, 0:4], ones128[:, 0:1], parts[:, 0:4],
                             start=True, stop=True)
            small = T(st, [1, 8], f"small{it}")
            nc.scalar.copy(small[0:1, 0:4], psr[0:1, 0:4])
            nc.sync.dma_start(out=ccin[it][0:1, 0:4], in_=small[0:1, 0:4])
            if use_cc:
                nc.gpsimd.collective_compute(
                    "AllReduce", OP.add, replica_groups=group,
                    ins=[ccin[it][:]], outs=[ccout[it][:]])
                src_cc = ccout[it]
            else:
                src_cc = ccin[it]
            bc = T(st, [128, 8], f"bc{it}")
            nc.sync.dma_start(out=bc, in_=src_cc[:].to_broadcast([128, 8]))

            # scalar recurrences on [128,1] replicated values
            rho = bc[:, 0:1]
            nu = bc[:, 2:3]
            mu_acc = T(st, [128, 1], f"mua{it}")
            nc.vector.tensor_tensor(mu_acc, bc[:, 1:2], bc[:, 3:4], op=OP.add)
            mu = T(st, [128, 1], f"mu{it}")
            nc.vector.scalar_tensor_tensor(
                mu, rho, lam128[:], mu_acc, op0=OP.mult, op1=OP.add)
            if it == 0:
                beta = zero1
                dd = mu
            else:
                rro = T(st, [128, 1], f"rro{it}")
                nc.vector.reciprocal(rro, rho_old)
                beta = T(st, [128, 1], f"beta{it}")
                nc.vector.tensor_tensor(beta, rro, rho, op=OP.mult)
                # d = mu + beta*(2*nu + beta*d_old)
                q1 = T(st, [128, 1], f"q1{it}")
                nc.vector.tensor_tensor(q1, beta, d_old, op=OP.mult)
                q2 = T(st, [128, 1], f"q2{it}")
                nc.vector.scalar_tensor_tensor(
                    q2, nu, 2.0, q1, op0=OP.mult, op1=OP.add)
                q3 = T(st, [128, 1], f"q3{it}")
                nc.vector.tensor_tensor(q3, beta, q2, op=OP.mult)
                dd = T(st, [128, 1], f"dd{it}")
                nc.vector.tensor_tensor(dd, q3, mu, op=OP.add)
            rrd = T(st, [128, 1], f"rrd{it}")
            nc.vector.reciprocal(rrd, dd)
            alpha = T(st, [128, 1], f"al{it}")
            nc.vector.tensor_tensor(alpha, rrd, rho, op=OP.mult)
            nalpha = T(st, [128, 1], f"nal{it}")
            nc.vector.tensor_scalar_mul(nalpha, alpha, -1.0)
            nab = T(st, [128, 1], f"nab{it}")
            nc.vector.tensor_tensor(nab, nalpha, beta, op=OP.mult)
            rho_old = rho
            d_old = dd

            if it == n_cg - 1:
                # final p, x updates: p10 = r10 + beta10*p9 ; x11 += a10*p10
                nc.vector.scalar_tensor_tensor(
                    p2, p2, beta[:], r_cur, op0=OP.mult, op1=OP.add)
                nc.vector.scalar_tensor_tensor(
                    x2, p2, alpha[:], x2, op0=OP.mult, op1=OP.add)

        # ---- output: x tall-split -> natural [2, 256*256] ----
        for i in range(2):
            nc.sync.dma_start(
                out=dout[i].rearrange("(t p w) -> p t w", t=2, p=128, w=256),
                in_=x2[:, i * 512:(i + 1) * 512].rearrange(
                    "p (t w) -> p t w", t=2))

    if evsem:
        import bass_rust as _bass_rust
        _bass_rust.generate_event_semaphores(nc)
        mybir.codegen_inst_isa_subclasses(nc)
    return nc


# ------------------------------------------------------------------ runner

_CACHE = {}


def _get_nc(key=(N_CG, NCOIL, N_CORES, True)):
    if key not in _CACHE:
        _CACHE[key] = build_nc(*key)
    return _CACHE[key]


def make_in_maps(inputs):
    shared = _prep_shared(
        inputs["w1"], inputs["b1"], inputs["w2"], inputs["b2"], inputs["w3"],
        inputs["b3"], inputs["w4"], inputs["b4"], inputs["w5"], inputs["b5"],
        inputs["lam"])
    in_maps = []
    for b in range(N_CORES):
        m = dict(shared)
        m.update(_prep_core(
            np.asarray(inputs["atb"][b], np.float32),
            np.asarray(inputs["csm_real"][b], np.float32),
            np.asarray(inputs["csm_imag"][b], np.float32),
            np.asarray(inputs["mask"][b], np.float32)))
        in_maps.append(m)
    return in_maps


def run(inputs, trace=False, **kw):
    nc = _get_nc()
    in_maps = make_in_maps(inputs)
    res = run_bass_kernel_spmd(nc, in_maps, core_ids=list(range(N_CORES)),
                               trace=trace, **kw)
    out = np.stack([np.asarray(r["out"]).reshape(2, 256, 256)
                    for r in res.results]).astype(np.float32)
    return out, res


def kernel(**inputs):
    out, _ = run(inputs, trace=False)
    return out
